# revision 12
# baseline (speedup 1.0000x reference)
"""Grouped-query attention, tensor-parallel over heads across 8 TRN2 NeuronCores.

Problem (hardcoded): x[1,1024,4096] @ Wq/Wk/Wv -> RoPE -> causal GQA
(32 q heads, 8 kv groups, head_dim 128) -> out proj Wo -> [1,1024,4096].

Sharding: core r owns q heads 4r..4r+3 and kv group r (Wq/Wk/Wv column
shards, Wo row shard). Each core computes a full [1024,4096] partial of
the output projection; the host sums the 8 partials (the "all-reduce").

Device kernel (per core): the big GEMMs (Q/K/V projections, out-proj)
run in fp8e4 DoubleRow mode, which processes two 128-deep contraction
chunks per instruction at half the cycles/row of bf16.  Precision is
recovered with a 3-term hi/lo split quantization (x_hi@W_hi + x_lo@W_hi
+ x_hi@W_lo), where hi and lo shares one power-of-2 scale so all terms
accumulate in a single PSUM chain; measured end-to-end error matches
bf16.  The attention core (scores, exp, denominators, ctx) stays bf16
with 256-wide s-blocks and causal tile skipping.
"""

import numpy as np
import ml_dtypes

import concourse.bass as bass
import concourse.bacc as bacc
import concourse.mybir as mybir
import concourse.tile as tile
from concourse.bass_utils import run_bass_kernel_spmd

S = 1024          # sequence length
D = 4096          # model dim
H = 32            # query heads (global)
G = 8             # kv groups (global)
HD = 128          # head dim
N_CORES = 8
HPC = H // N_CORES   # 4 query heads per core
QW = HPC * HD        # 512 q-proj cols per core
NDC = D // 128       # 32 contraction chunks
NP = NDC // 2        # 16 DoubleRow chunk pairs
BF = mybir.dt.bfloat16
F8 = mybir.dt.float8e4
F32 = mybir.dt.float32
DR = mybir.MatmulPerfMode.DoubleRow

# quantization scales (powers of 2; hi and lo share the scale so every
# 3-term matmul accumulates in one PSUM chain)
XS = 16.0
WQS = 8192.0        # applied to Wq/sqrt(HD)
WKS = 1024.0
WVS = 1024.0
WOS = 1024.0
CTXS = 16.0
EXP_SHIFT = -6.0    # exp(s - 6): keeps bf16 P comfortably in range

_CACHE = {}


def _build():
    nc = bacc.Bacc("TRN2", target_bir_lowering=False, debug=False,
                   num_devices=N_CORES)

    xh = nc.dram_tensor("xh", [128, NDC, S], F8, kind="ExternalInput")
    xl = nc.dram_tensor("xl", [128, NDC, S], F8, kind="ExternalInput")
    wqh = nc.dram_tensor("wqh", [128, NDC, QW], F8, kind="ExternalInput")
    wql = nc.dram_tensor("wql", [128, NDC, QW], F8, kind="ExternalInput")
    wkh = nc.dram_tensor("wkh", [128, NDC, HD], F8, kind="ExternalInput")
    wkl = nc.dram_tensor("wkl", [128, NDC, HD], F8, kind="ExternalInput")
    wvh = nc.dram_tensor("wvh", [128, NDC, HD], F8, kind="ExternalInput")
    wvl = nc.dram_tensor("wvl", [128, NDC, HD], F8, kind="ExternalInput")
    woh = nc.dram_tensor("woh", [128, HPC, D], F8, kind="ExternalInput")
    wol = nc.dram_tensor("wol", [128, HPC, D], F8, kind="ExternalInput")
    cosT = nc.dram_tensor("cosT", [HD, S], BF, kind="ExternalInput")
    sinT = nc.dram_tensor("sinT", [HD, S], BF, kind="ExternalInput")
    rmat = nc.dram_tensor("rmat", [HD, HD], BF, kind="ExternalInput")
    masks = nc.dram_tensor("masks", [128, 512], BF, kind="ExternalInput")
    out = nc.dram_tensor("out", [S, D], BF, kind="ExternalOutput")

    with tile.TileContext(nc) as tc:
        _emit(tc, nc, xh, xl, wqh, wql, wkh, wkl, wvh, wvl, woh, wol,
              cosT, sinT, rmat, masks, out)
    nc.compile()
    return nc


def _emit(tc, nc, xh, xl, wqh, wql, wkh, wkl, wvh, wvl, woh, wol,
          cosT, sinT, rmat, masks, out):
    import contextlib
    ctx = contextlib.ExitStack()
    with ctx:
        const = ctx.enter_context(tc.tile_pool(name="const", bufs=1))
        work = ctx.enter_context(tc.tile_pool(name="work", bufs=1))
        tmp = ctx.enter_context(tc.tile_pool(name="tmp", bufs=4))
        pt_pool = ctx.enter_context(tc.tile_pool(name="pt", bufs=8))
        outp = ctx.enter_context(tc.tile_pool(name="outp", bufs=3))
        ps = ctx.enter_context(tc.tile_pool(name="ps", bufs=8, space="PSUM"))

        # ---- DMA emission, ordered to pace the chunk-major PE stream ----
        rmat_sb = const.tile([HD, HD], BF, tag="rmat")
        ones_sb = const.tile([128, 1], BF, tag="ones")
        nc.vector.memset(ones_sb[:], 1.0)
        ebias = const.tile([128, 1], F32, tag="ebias")
        nc.vector.memset(ebias[:], EXP_SHIFT)

        wk_sb = const.tile([128, 2, NDC, HD], F8, tag="wk")   # dim1: hi/lo
        nc.sync.dma_start(out=rmat_sb[:], in_=rmat.ap())

        gx4h, gx4l = {}, {}
        gqh, gql = {}, {}
        for c in range(0, NDC, 4):
            gx4h[c] = const.tile([128, 4, S], F8, tag=f"xh{c//4}", name=f"xh{c//4}")
            gx4l[c] = const.tile([128, 4, S], F8, tag=f"xl{c//4}", name=f"xl{c//4}")
        gxh = {c: gx4h[c - c % 4][:, c % 4:c % 4 + 2, :] for c in range(0, NDC, 2)}
        gxl = {c: gx4l[c - c % 4][:, c % 4:c % 4 + 2, :] for c in range(0, NDC, 2)}
        # half-0 of x, wq, and wk interleaved in consumption order
        for c in range(0, NDC, 4):
            nc.sync.dma_start(out=wk_sb[:, 0, c:c + 4, :], in_=wkh.ap()[:, c:c + 4, :])
            g = const.tile([128, 4, QW], F8, tag=f"qh{c//4}", name=f"qh{c//4}")
            nc.sync.dma_start(out=g[:], in_=wqh.ap()[:, c:c + 4, :])
            gqh[c], gqh[c + 2] = g[:, 0:2, :], g[:, 2:4, :]
            nc.sync.dma_start(out=gx4h[c][:, :, 0:512], in_=xh.ap()[:, c:c + 4, 0:512])
            nc.sync.dma_start(out=wk_sb[:, 1, c:c + 4, :], in_=wkl.ap()[:, c:c + 4, :])
            g = const.tile([128, 4, QW], F8, tag=f"ql{c//4}", name=f"ql{c//4}")
            nc.sync.dma_start(out=g[:], in_=wql.ap()[:, c:c + 4, :])
            gql[c], gql[c + 2] = g[:, 0:2, :], g[:, 2:4, :]
            nc.sync.dma_start(out=gx4l[c][:, :, 0:512], in_=xl.ap()[:, c:c + 4, 0:512])
            if c == 0:
                wv_sb = const.tile([128, 2, NDC, HD], F8, tag="wv")
                nc.sync.dma_start(out=wv_sb[:, 0, :, :], in_=wvh.ap())
            if c == 4:
                nc.sync.dma_start(out=wv_sb[:, 1, :, :], in_=wvl.ap())
        cos_sb = const.tile([HD, S], BF, tag="cos")
        nc.sync.dma_start(out=cos_sb[:], in_=cosT.ap())
        sin_sb = const.tile([HD, S], BF, tag="sin")
        nc.sync.dma_start(out=sin_sb[:], in_=sinT.ap())
        # half-1 of x
        for c in range(0, NDC, 4):
            nc.sync.dma_start(out=gx4h[c][:, :, 512:S], in_=xh.ap()[:, c:c + 4, 512:S])
            nc.sync.dma_start(out=gx4l[c][:, :, 512:S], in_=xl.ap()[:, c:c + 4, 512:S])
        mask_sb = const.tile([128, 512], BF, tag="mask")
        nc.sync.dma_start(out=mask_sb[:], in_=masks.ap())
        wo_sb = const.tile([128, 2, HPC, D], F8, tag="wo")    # dim1: hi/lo
        for n in range(2):
            sl = slice(n * 2048, (n + 1) * 2048)
            nc.sync.dma_start(out=wo_sb[:, 0, :, sl], in_=woh.ap()[:, :, sl])
            nc.sync.dma_start(out=wo_sb[:, 1, :, sl], in_=wol.ap()[:, :, sl])

        # persistent activations
        khat = work.tile([HD, S], BF, tag="khat")
        qhat = [work.tile([HD, S], BF, tag=f"qhat{h}", name=f"qhat{h}")
                for h in range(HPC)]
        v_sb = [work.tile([128, HD], BF, tag=f"v{i}", name=f"v{i}")
                for i in range(8)]
        ctx_hi = [work.tile([128, 2, S], F8, tag=f"cth{u}", name=f"cth{u}")
                  for u in range(2)]
        ctx_lo = [work.tile([128, 2, S], F8, tag=f"ctl{u}", name=f"ctl{u}")
                  for u in range(2)]

        # ---- K+Q projections: chunk-major across 5 chains per s-quarter ----
        # Per chunk pair, all five tensors advance their 3-term DoubleRow
        # chains, so the PE stream follows the x/wq DMA arrival order.
        # RoPE for each finished s-half is queued and its PE/Act/DVE work is
        # injected into later quarters' streams (and the v-projection).
        TENS = [("k", khat, lambda c: wk_sb[:, 0, c:c + 2, :],
                 lambda c: wk_sb[:, 1, c:c + 2, :], 1.0 / (XS * WKS))]
        for h in range(HPC):
            hsl = slice(h * HD, (h + 1) * HD)
            TENS.append((f"q{h}", qhat[h],
                         lambda c, s=hsl: gqh[c][:, :, s],
                         lambda c, s=hsl: gql[c][:, :, s], 1.0 / (XS * WQS)))
        raws = {ti: work.tile([HD, S], BF, tag=f"raw{ti}", name=f"raw{ti}")
                for ti in range(5)}

        pend = []   # queued rope-finish closures (one per (tensor, half))

        def inject_rope():
            if pend:
                pend.pop(0)()

        def rope_half(ti, half):
            name, dst, _, _, _ = TENS[ti]
            sl = slice(half * 512, (half + 1) * 512)
            t1 = tmp.tile([HD, 512], BF, tag="rope_t1", name="rope_t1", bufs=2)
            nc.vector.tensor_mul(t1[:], raws[ti][:, sl], cos_sb[:, sl])
            rq = ps.tile([HD, 512], F32, tag="ps", name="rq")
            nc.tensor.matmul(rq[:], rmat_sb[:], raws[ti][:, sl],
                             start=True, stop=True)
            rqs = tmp.tile([HD, 512], BF, tag="rope_rqs", name="rope_rqs", bufs=2)
            nc.scalar.activation(rqs[:], rq[:],
                                 mybir.ActivationFunctionType.Copy)
            t2 = tmp.tile([HD, 512], BF, tag="rope_t2", name="rope_t2", bufs=2)
            nc.vector.tensor_mul(t2[:], rqs[:], sin_sb[:, sl])
            nc.vector.tensor_add(dst[:, sl], t1[:], t2[:])

        def v_terms(vpsum, i, p):
            c = 2 * p
            tsl = slice(i * 128, (i + 1) * 128)
            nc.tensor.matmul(vpsum[:], gxh[c][:, :, tsl], wv_sb[:, 0, c:c + 2, :],
                             start=(p == 0), stop=False,
                             perf_mode=DR, skip_group_check=True)
            nc.tensor.matmul(vpsum[:], gxh[c][:, :, tsl], wv_sb[:, 1, c:c + 2, :],
                             start=False, stop=False,
                             perf_mode=DR, skip_group_check=True)
            nc.tensor.matmul(vpsum[:], gxl[c][:, :, tsl], wv_sb[:, 0, c:c + 2, :],
                             start=False, stop=(p == NP - 1),
                             perf_mode=DR, skip_group_check=True)

        for half in (0, 1):
            hoff = half * 512
            chains = [ps.tile([128, 512], F32, tag="ps", name=f"ch{ti}")
                      for ti in range(5)]
            if half == 0:
                vps = [ps.tile([128, HD], F32, tag="ps", name=f"vps{i}")
                       for i in range(2)]
            for p in range(NP):
                c = 2 * p
                if half == 0:
                    for i in range(2):
                        v_terms(vps[i], i, p)
                for term in range(3):
                    for ti, (_, _, whi, wlo, _) in enumerate(TENS):
                        pp = chains[ti]
                        w = whi(c) if term != 1 else wlo(c)
                        for q in range(2):
                            ssl = slice(hoff + q * 256, hoff + (q + 1) * 256)
                            osl = slice(q * 256, (q + 1) * 256)
                            xop = gxl[c] if term == 2 else gxh[c]
                            nc.tensor.matmul(
                                pp[:, osl], w, xop[:, :, ssl],
                                start=(p == 0 and term == 0 and q == 0),
                                stop=(p == NP - 1 and term == 2 and q == 1),
                                perf_mode=DR, skip_group_check=True)
                if p in (4, 9, 14):
                    inject_rope()
            for ti, (_, _, _, _, descale) in enumerate(TENS):
                nc.scalar.activation(raws[ti][:, hoff:hoff + 512], chains[ti][:],
                                     mybir.ActivationFunctionType.Copy,
                                     scale=descale)
            if half == 0:
                for i in range(2):
                    nc.scalar.activation(v_sb[i][:], vps[i][:],
                                         mybir.ActivationFunctionType.Copy,
                                         scale=1.0 / (XS * WVS))
            for ti in range(5):
                pend.append(lambda t=ti, hf=half: rope_half(t, hf))

        # ---- V projection: emitted as PE filler inside early attention ----
        def v_chain(i):
            tsl = slice(i * 128, (i + 1) * 128)
            vp = ps.tile([128, HD], F32, tag="ps", name="vp")
            for p in range(NP):
                c = 2 * p
                nc.tensor.matmul(vp[:], gxh[c][:, :, tsl], wv_sb[:, 0, c:c + 2, :],
                                 start=(p == 0), stop=False, perf_mode=DR)
            for p in range(NP):
                c = 2 * p
                nc.tensor.matmul(vp[:], gxh[c][:, :, tsl], wv_sb[:, 1, c:c + 2, :],
                                 start=False, stop=False, perf_mode=DR)
            for p in range(NP):
                c = 2 * p
                nc.tensor.matmul(vp[:], gxl[c][:, :, tsl], wv_sb[:, 0, c:c + 2, :],
                                 start=False, stop=(p == NP - 1), perf_mode=DR)
            nc.scalar.activation(v_sb[i][:], vp[:],
                                 mybir.ActivationFunctionType.Copy,
                                 scale=1.0 / (XS * WVS))
            inject_rope()

        vq = list(range(2, 8))
        while pend and len(vq) > 4:
            v_chain(vq.pop(0))
        while pend:
            inject_rope()

        # ---- attention + out-proj, software-pipelined ----
        def emit_scores(h, b):
            ssl = slice(b * 256, (b + 1) * 256)
            pts = []
            for tp in range(b + 1):
                st = ps.tile([128, 512], F32, tag="ps", name="st")
                for i in range(2):
                    t0 = (2 * tp + i) * 128
                    nc.tensor.matmul(st[:, i * 256:(i + 1) * 256],
                                     khat[:, t0:t0 + 128], qhat[h][:, ssl],
                                     start=True, stop=True)
                pt = pt_pool.tile([128, 512], BF, tag="pt", name="pt")
                nc.scalar.activation(pt[:], st[:],
                                     mybir.ActivationFunctionType.Exp,
                                     bias=ebias[:])
                if tp == b:
                    nc.vector.tensor_mul(pt[:], pt[:], mask_sb[:])
                pts.append(pt)
            return pts

        def emit_denctx(h, b, pts):
            ssl = slice(b * 256, (b + 1) * 256)
            den = ps.tile([1, 256], F32, tag="ps", name="den")
            n_mm = 2 * (b + 1)
            k = 0
            for pt in pts:
                for i in range(2):
                    nc.tensor.matmul(den[:], ones_sb[:],
                                     pt[:, i * 256:(i + 1) * 256],
                                     start=(k == 0), stop=(k == n_mm - 1))
                    k += 1
            cx = ps.tile([HD, 256], F32, tag="ps", name="cx")
            k = 0
            for tp, pt in enumerate(pts):
                for i in range(2):
                    nc.tensor.matmul(cx[:], v_sb[2 * tp + i][:],
                                     pt[:, i * 256:(i + 1) * 256],
                                     start=(k == 0), stop=(k == n_mm - 1))
                    k += 1
            rec = tmp.tile([1, 256], F32, tag="rec", name="rec", bufs=2)
            nc.vector.reciprocal(rec[:], den[:])
            bc = tmp.tile([128, 256], F32, tag="bc", name="bc", bufs=2)
            nc.gpsimd.partition_broadcast(bc[:], rec[:])
            ctxn = tmp.tile([HD, 256], F32, tag="ctxn", name="ctxn", bufs=2)
            nc.vector.scalar_tensor_tensor(
                ctxn[:], cx[:], CTXS, bc[:],
                op0=mybir.AluOpType.mult, op1=mybir.AluOpType.mult)
            u, par = divmod(h, 2)
            nc.scalar.activation(ctx_hi[u][:, par, ssl], ctxn[:],
                                 mybir.ActivationFunctionType.Copy)
            nc.vector.tensor_sub(ctx_lo[u][:, par, ssl], ctxn[:],
                                 ctx_hi[u][:, par, ssl])

        descale = 1.0 / (CTXS * WOS)

        def emit_outproj(t8):
            tsl = slice(t8 * 128, (t8 + 1) * 128)
            for n4 in range(4):
                ot = outp.tile([128, 1024], BF, tag="ot", name="ot")
                for sub in range(4):
                    n = 4 * n4 + sub
                    nsl = slice(n * 256, (n + 1) * 256)
                    op = ps.tile([128, 256], F32, tag="ps", name="op")
                    k = 0
                    for u in range(2):
                        for chi, whi in ((ctx_hi, 0), (ctx_lo, 0), (ctx_hi, 1)):
                            nc.tensor.matmul(
                                op[:], chi[u][:, :, tsl],
                                wo_sb[:, whi, 2 * u:2 * u + 2, nsl],
                                start=(k == 0), stop=(k == 5), perf_mode=DR)
                            k += 1
                    nc.gpsimd.tensor_scalar_mul(ot[:, sub * 256:(sub + 1) * 256],
                                                op[:], descale)
                nc.sync.dma_start(
                    out=out.ap()[tsl, n4 * 1024:(n4 + 1) * 1024], in_=ot[:])

        stages = [(h, b) for b in range(4) for h in range(HPC)]
        prev = None
        outq = []
        for hb in stages:
            pts = emit_scores(*hb)
            if vq:
                v_chain(vq.pop(0))
            if prev is not None:
                (ph, pb), ppts = prev
                emit_denctx(ph, pb, ppts)
                if ph == HPC - 1:
                    outq.extend([2 * pb, 2 * pb + 1])
            if outq:
                emit_outproj(outq.pop(0))
            prev = (hb, pts)
        (ph, pb), ppts = prev
        emit_denctx(ph, pb, ppts)
        outq.extend([2 * pb, 2 * pb + 1])
        for t8 in outq:
            emit_outproj(t8)


def _prep_inputs(x, cos, sin, Wq, Wk, Wv, Wo):
    """Host-side shard + hi/lo fp8 quantization. Returns per-core inputs."""
    bf = ml_dtypes.bfloat16
    f8 = ml_dtypes.float8_e4m3

    def hilo(a, s):
        hi = np.asarray(a * s, np.float32).astype(f8)
        lo = (np.asarray(a * s, np.float32) - hi.astype(np.float32)).astype(f8)
        return hi, lo

    x2 = np.asarray(x, np.float32).reshape(S, D)
    xTh = np.ascontiguousarray(x2.T).reshape(NDC, 128, S).transpose(1, 0, 2)
    xh_, xl_ = hilo(np.ascontiguousarray(xTh), XS)

    cosT = np.ascontiguousarray(np.asarray(cos, np.float32).T).astype(bf)
    sinT = np.ascontiguousarray(np.asarray(sin, np.float32).T).astype(bf)

    rmat = np.zeros((HD, HD), np.float32)
    half = HD // 2
    rmat[np.arange(half), np.arange(half) + half] = 1.0
    rmat[np.arange(half) + half, np.arange(half)] = -1.0
    rmat = rmat.astype(bf)

    # diagonal pair mask: keep when t_local (= i*128 + p) <= s_local
    lt = np.arange(128)[:, None]
    ls = np.arange(256)[None, :]
    masks = np.concatenate([(lt + 128 * i <= ls) for i in range(2)], axis=1)
    masks = np.ascontiguousarray(masks).astype(bf)     # [128, 512]

    scale = 1.0 / np.sqrt(np.float32(HD))
    Wq_ = np.asarray(Wq, np.float32) * scale
    Wk_ = np.asarray(Wk, np.float32)
    Wv_ = np.asarray(Wv, np.float32)
    Wo_ = np.asarray(Wo, np.float32)

    def chunked(w):  # [D, m] -> [128, NDC, m]
        m = w.shape[1]
        return np.ascontiguousarray(
            w.reshape(NDC, 128, m).transpose(1, 0, 2))

    in_maps = []
    for r in range(N_CORES):
        wqh_, wql_ = hilo(chunked(Wq_[:, r * QW:(r + 1) * QW]), WQS)
        wkh_, wkl_ = hilo(chunked(Wk_[:, r * HD:(r + 1) * HD]), WKS)
        wvh_, wvl_ = hilo(chunked(Wv_[:, r * HD:(r + 1) * HD]), WVS)
        wo_r = np.ascontiguousarray(
            Wo_[r * QW:(r + 1) * QW, :].reshape(HPC, 128, D)
            .transpose(1, 0, 2))
        woh_, wol_ = hilo(wo_r, WOS)
        in_maps.append({
            "xh": xh_, "xl": xl_, "wqh": wqh_, "wql": wql_,
            "wkh": wkh_, "wkl": wkl_, "wvh": wvh_, "wvl": wvl_,
            "woh": woh_, "wol": wol_,
            "cosT": cosT, "sinT": sinT, "rmat": rmat, "masks": masks,
        })
    return in_maps


def get_nc():
    if "nc" not in _CACHE:
        _CACHE["nc"] = _build()
    return _CACHE["nc"]


def kernel(x, mask, cos, sin, Wq, Wk, Wv, Wo):
    nc = get_nc()
    in_maps = _prep_inputs(x, cos, sin, Wq, Wk, Wv, Wo)
    res = run_bass_kernel_spmd(nc, in_maps, core_ids=list(range(N_CORES)))
    acc = np.zeros((S, D), np.float32)
    for r in range(N_CORES):
        acc += res.results[r]["out"].astype(np.float32)
    return acc[None]


if __name__ == "__main__":
    print("built:", get_nc() is not None)


# revision 13
# speedup vs baseline: 1.0117x; 1.0117x over previous
"""Grouped-query attention, tensor-parallel over heads across 8 TRN2 NeuronCores.

Problem (hardcoded): x[1,1024,4096] @ Wq/Wk/Wv -> RoPE -> causal GQA
(32 q heads, 8 kv groups, head_dim 128) -> out proj Wo -> [1,1024,4096].

Sharding: core r owns q heads 4r..4r+3 and kv group r (Wq/Wk/Wv column
shards, Wo row shard). Each core computes a full [1024,4096] partial of
the output projection; the host sums the 8 partials (the "all-reduce").

Device kernel (per core): the big GEMMs (Q/K/V projections, out-proj)
run in fp8e4 DoubleRow mode, which processes two 128-deep contraction
chunks per instruction at half the cycles/row of bf16.  Precision is
recovered with a 3-term hi/lo split quantization (x_hi@W_hi + x_lo@W_hi
+ x_hi@W_lo), where hi and lo shares one power-of-2 scale so all terms
accumulate in a single PSUM chain; measured end-to-end error matches
bf16.  The attention core (scores, exp, denominators, ctx) stays bf16
with 256-wide s-blocks and causal tile skipping.
"""

import numpy as np
import ml_dtypes

import concourse.bass as bass
import concourse.bacc as bacc
import concourse.mybir as mybir
import concourse.tile as tile
from concourse.bass_utils import run_bass_kernel_spmd

S = 1024          # sequence length
D = 4096          # model dim
H = 32            # query heads (global)
G = 8             # kv groups (global)
HD = 128          # head dim
N_CORES = 8
HPC = H // N_CORES   # 4 query heads per core
QW = HPC * HD        # 512 q-proj cols per core
NDC = D // 128       # 32 contraction chunks
NP = NDC // 2        # 16 DoubleRow chunk pairs
BF = mybir.dt.bfloat16
F8 = mybir.dt.float8e4
F32 = mybir.dt.float32
DR = mybir.MatmulPerfMode.DoubleRow

# quantization scales (powers of 2; hi and lo share the scale so every
# 3-term matmul accumulates in one PSUM chain)
XS = 16.0
WQS = 8192.0        # applied to Wq/sqrt(HD)
WKS = 1024.0
WVS = 1024.0
WOS = 1024.0
CTXS = 16.0
EXP_SHIFT = -6.0    # exp(s - 6): keeps bf16 P comfortably in range

_CACHE = {}


def _build():
    nc = bacc.Bacc("TRN2", target_bir_lowering=False, debug=False,
                   num_devices=N_CORES)

    xh = nc.dram_tensor("xh", [128, NDC, S], F8, kind="ExternalInput")
    xl = nc.dram_tensor("xl", [128, NDC, S], F8, kind="ExternalInput")
    wqh = nc.dram_tensor("wqh", [128, NDC, QW], F8, kind="ExternalInput")
    wql = nc.dram_tensor("wql", [128, NDC, QW], F8, kind="ExternalInput")
    wkh = nc.dram_tensor("wkh", [128, NDC, HD], F8, kind="ExternalInput")
    wkl = nc.dram_tensor("wkl", [128, NDC, HD], F8, kind="ExternalInput")
    wvh = nc.dram_tensor("wvh", [128, NDC, HD], F8, kind="ExternalInput")
    wvl = nc.dram_tensor("wvl", [128, NDC, HD], F8, kind="ExternalInput")
    woh = nc.dram_tensor("woh", [128, HPC, D], F8, kind="ExternalInput")
    wol = nc.dram_tensor("wol", [128, HPC, D], F8, kind="ExternalInput")
    cosT = nc.dram_tensor("cosT", [HD, S], BF, kind="ExternalInput")
    sinT = nc.dram_tensor("sinT", [HD, S], BF, kind="ExternalInput")
    rmat = nc.dram_tensor("rmat", [HD, HD], BF, kind="ExternalInput")
    masks = nc.dram_tensor("masks", [128, 512], BF, kind="ExternalInput")
    out = nc.dram_tensor("out", [S, D], BF, kind="ExternalOutput")

    with tile.TileContext(nc) as tc:
        _emit(tc, nc, xh, xl, wqh, wql, wkh, wkl, wvh, wvl, woh, wol,
              cosT, sinT, rmat, masks, out)
    nc.compile()
    return nc


def _emit(tc, nc, xh, xl, wqh, wql, wkh, wkl, wvh, wvl, woh, wol,
          cosT, sinT, rmat, masks, out):
    import contextlib
    ctx = contextlib.ExitStack()
    with ctx:
        const = ctx.enter_context(tc.tile_pool(name="const", bufs=1))
        work = ctx.enter_context(tc.tile_pool(name="work", bufs=1))
        tmp = ctx.enter_context(tc.tile_pool(name="tmp", bufs=4))
        pt_pool = ctx.enter_context(tc.tile_pool(name="pt", bufs=8))
        outp = ctx.enter_context(tc.tile_pool(name="outp", bufs=3))
        ps = ctx.enter_context(tc.tile_pool(name="ps", bufs=8, space="PSUM"))

        # ---- DMA emission, ordered to pace the chunk-major PE stream ----
        rmat_sb = const.tile([HD, HD], BF, tag="rmat")
        ones_sb = const.tile([128, 1], BF, tag="ones")
        nc.vector.memset(ones_sb[:], 1.0)
        ebias = const.tile([128, 1], F32, tag="ebias")
        nc.vector.memset(ebias[:], EXP_SHIFT)

        wk_sb = const.tile([128, 2, NDC, HD], F8, tag="wk")   # dim1: hi/lo
        nc.sync.dma_start(out=rmat_sb[:], in_=rmat.ap())

        gx4h, gx4l = {}, {}
        gqh, gql = {}, {}
        for c in range(0, NDC, 4):
            gx4h[c] = const.tile([128, 4, S], F8, tag=f"xh{c//4}", name=f"xh{c//4}")
            gx4l[c] = const.tile([128, 4, S], F8, tag=f"xl{c//4}", name=f"xl{c//4}")
        gxh = {c: gx4h[c - c % 4][:, c % 4:c % 4 + 2, :] for c in range(0, NDC, 2)}
        gxl = {c: gx4l[c - c % 4][:, c % 4:c % 4 + 2, :] for c in range(0, NDC, 2)}
        # half-0 of x, wq, and wk interleaved in consumption order
        for c in range(0, NDC, 4):
            nc.sync.dma_start(out=wk_sb[:, 0, c:c + 4, :], in_=wkh.ap()[:, c:c + 4, :])
            g = const.tile([128, 4, QW], F8, tag=f"qh{c//4}", name=f"qh{c//4}")
            nc.sync.dma_start(out=g[:], in_=wqh.ap()[:, c:c + 4, :])
            gqh[c], gqh[c + 2] = g[:, 0:2, :], g[:, 2:4, :]
            nc.sync.dma_start(out=gx4h[c][:, :, 0:512], in_=xh.ap()[:, c:c + 4, 0:512])
            nc.sync.dma_start(out=wk_sb[:, 1, c:c + 4, :], in_=wkl.ap()[:, c:c + 4, :])
            g = const.tile([128, 4, QW], F8, tag=f"ql{c//4}", name=f"ql{c//4}")
            nc.sync.dma_start(out=g[:], in_=wql.ap()[:, c:c + 4, :])
            gql[c], gql[c + 2] = g[:, 0:2, :], g[:, 2:4, :]
            nc.sync.dma_start(out=gx4l[c][:, :, 0:512], in_=xl.ap()[:, c:c + 4, 0:512])
            if c == 0:
                wv_sb = const.tile([128, 2, NDC, HD], F8, tag="wv")
                nc.sync.dma_start(out=wv_sb[:, 0, :, :], in_=wvh.ap())
            if c == 4:
                nc.sync.dma_start(out=wv_sb[:, 1, :, :], in_=wvl.ap())
        cos_sb = const.tile([HD, S], BF, tag="cos")
        nc.sync.dma_start(out=cos_sb[:], in_=cosT.ap())
        sin_sb = const.tile([HD, S], BF, tag="sin")
        nc.sync.dma_start(out=sin_sb[:], in_=sinT.ap())
        # half-1 of x
        for c in range(0, NDC, 4):
            nc.sync.dma_start(out=gx4h[c][:, :, 512:S], in_=xh.ap()[:, c:c + 4, 512:S])
            nc.sync.dma_start(out=gx4l[c][:, :, 512:S], in_=xl.ap()[:, c:c + 4, 512:S])
        mask_sb = const.tile([128, 512], BF, tag="mask")
        nc.sync.dma_start(out=mask_sb[:], in_=masks.ap())
        wo_sb = const.tile([128, 2, HPC, D], F8, tag="wo")    # dim1: hi/lo
        for n in range(2):
            sl = slice(n * 2048, (n + 1) * 2048)
            nc.sync.dma_start(out=wo_sb[:, 0, :, sl], in_=woh.ap()[:, :, sl])
            nc.sync.dma_start(out=wo_sb[:, 1, :, sl], in_=wol.ap()[:, :, sl])

        # persistent activations
        khat = work.tile([HD, S], BF, tag="khat")
        qhat = [work.tile([HD, S], BF, tag=f"qhat{h}", name=f"qhat{h}")
                for h in range(HPC)]
        v_sb = [work.tile([128, HD], BF, tag=f"v{i}", name=f"v{i}")
                for i in range(8)]
        ctx_hi = [work.tile([128, 2, S], F8, tag=f"cth{u}", name=f"cth{u}")
                  for u in range(2)]
        ctx_lo = [work.tile([128, 2, S], F8, tag=f"ctl{u}", name=f"ctl{u}")
                  for u in range(2)]

        # ---- K+Q projections: chunk-major across 5 chains per s-quarter ----
        # Per chunk pair, all five tensors advance their 3-term DoubleRow
        # chains, so the PE stream follows the x/wq DMA arrival order.
        # RoPE for each finished s-half is queued and its PE/Act/DVE work is
        # injected into later quarters' streams (and the v-projection).
        TENS = [("k", khat, lambda c: wk_sb[:, 0, c:c + 2, :],
                 lambda c: wk_sb[:, 1, c:c + 2, :], 1.0 / (XS * WKS))]
        for h in range(HPC):
            hsl = slice(h * HD, (h + 1) * HD)
            TENS.append((f"q{h}", qhat[h],
                         lambda c, s=hsl: gqh[c][:, :, s],
                         lambda c, s=hsl: gql[c][:, :, s], 1.0 / (XS * WQS)))
        raws = {ti: work.tile([HD, S], BF, tag=f"raw{ti}", name=f"raw{ti}")
                for ti in range(5)}

        pend = []   # queued rope-finish closures (one per (tensor, half))

        def inject_rope():
            if pend:
                pend.pop(0)()

        def rope_half(ti, half):
            name, dst, _, _, _ = TENS[ti]
            sl = slice(half * 512, (half + 1) * 512)
            t1 = tmp.tile([HD, 512], BF, tag="rope_t1", name="rope_t1", bufs=2)
            nc.vector.tensor_mul(t1[:], raws[ti][:, sl], cos_sb[:, sl])
            rq = ps.tile([HD, 512], F32, tag="ps", name="rq")
            nc.tensor.matmul(rq[:], rmat_sb[:], raws[ti][:, sl],
                             start=True, stop=True)
            rqs = tmp.tile([HD, 512], BF, tag="rope_rqs", name="rope_rqs", bufs=2)
            nc.scalar.activation(rqs[:], rq[:],
                                 mybir.ActivationFunctionType.Copy)
            t2 = tmp.tile([HD, 512], BF, tag="rope_t2", name="rope_t2", bufs=2)
            nc.vector.tensor_mul(t2[:], rqs[:], sin_sb[:, sl])
            nc.vector.tensor_add(dst[:, sl], t1[:], t2[:])

        def v_terms(vpsum, i, p):
            c = 2 * p
            tsl = slice(i * 128, (i + 1) * 128)
            nc.tensor.matmul(vpsum[:], gxh[c][:, :, tsl], wv_sb[:, 0, c:c + 2, :],
                             start=(p == 0), stop=False,
                             perf_mode=DR, skip_group_check=True)
            nc.tensor.matmul(vpsum[:], gxh[c][:, :, tsl], wv_sb[:, 1, c:c + 2, :],
                             start=False, stop=False,
                             perf_mode=DR, skip_group_check=True)
            nc.tensor.matmul(vpsum[:], gxl[c][:, :, tsl], wv_sb[:, 0, c:c + 2, :],
                             start=False, stop=(p == NP - 1),
                             perf_mode=DR, skip_group_check=True)

        for half in (0, 1):
            hoff = half * 512
            chains = [ps.tile([128, 512], F32, tag="ps", name=f"ch{ti}")
                      for ti in range(5)]
            for p in range(NP):
                c = 2 * p
                for term in range(3):
                    for ti, (_, _, whi, wlo, _) in enumerate(TENS):
                        pp = chains[ti]
                        w = whi(c) if term != 1 else wlo(c)
                        for q in range(2):
                            ssl = slice(hoff + q * 256, hoff + (q + 1) * 256)
                            osl = slice(q * 256, (q + 1) * 256)
                            xop = gxl[c] if term == 2 else gxh[c]
                            nc.tensor.matmul(
                                pp[:, osl], w, xop[:, :, ssl],
                                start=(p == 0 and term == 0 and q == 0),
                                stop=(p == NP - 1 and term == 2 and q == 1),
                                perf_mode=DR, skip_group_check=True)
                if p in (4, 9, 14):
                    inject_rope()
            for ti, (_, _, _, _, descale) in enumerate(TENS):
                nc.scalar.activation(raws[ti][:, hoff:hoff + 512], chains[ti][:],
                                     mybir.ActivationFunctionType.Copy,
                                     scale=descale)
            for ti in range(5):
                pend.append(lambda t=ti, hf=half: rope_half(t, hf))

        # ---- V projection: emitted as PE filler inside early attention ----
        def v_chain(i):
            tsl = slice(i * 128, (i + 1) * 128)
            vp = ps.tile([128, HD], F32, tag="ps", name="vp")
            for p in range(NP):
                c = 2 * p
                nc.tensor.matmul(vp[:], gxh[c][:, :, tsl], wv_sb[:, 0, c:c + 2, :],
                                 start=(p == 0), stop=False, perf_mode=DR)
            for p in range(NP):
                c = 2 * p
                nc.tensor.matmul(vp[:], gxh[c][:, :, tsl], wv_sb[:, 1, c:c + 2, :],
                                 start=False, stop=False, perf_mode=DR)
            for p in range(NP):
                c = 2 * p
                nc.tensor.matmul(vp[:], gxl[c][:, :, tsl], wv_sb[:, 0, c:c + 2, :],
                                 start=False, stop=(p == NP - 1), perf_mode=DR)
            nc.scalar.activation(v_sb[i][:], vp[:],
                                 mybir.ActivationFunctionType.Copy,
                                 scale=1.0 / (XS * WVS))
            inject_rope()

        v_chain(0)
        v_chain(1)
        vq = list(range(2, 8))
        while pend and len(vq) > 4:
            v_chain(vq.pop(0))
        while pend:
            inject_rope()

        # ---- attention + out-proj, software-pipelined ----
        def emit_scores(h, b):
            ssl = slice(b * 256, (b + 1) * 256)
            pts = []
            for tp in range(b + 1):
                st = ps.tile([128, 512], F32, tag="ps", name="st")
                for i in range(2):
                    t0 = (2 * tp + i) * 128
                    nc.tensor.matmul(st[:, i * 256:(i + 1) * 256],
                                     khat[:, t0:t0 + 128], qhat[h][:, ssl],
                                     start=True, stop=True)
                pt = pt_pool.tile([128, 512], BF, tag="pt", name="pt")
                nc.scalar.activation(pt[:], st[:],
                                     mybir.ActivationFunctionType.Exp,
                                     bias=ebias[:])
                if tp == b:
                    nc.vector.tensor_mul(pt[:], pt[:], mask_sb[:])
                pts.append(pt)
            return pts

        def emit_denctx(h, b, pts):
            ssl = slice(b * 256, (b + 1) * 256)
            den = ps.tile([1, 256], F32, tag="ps", name="den")
            n_mm = 2 * (b + 1)
            k = 0
            for pt in pts:
                for i in range(2):
                    nc.tensor.matmul(den[:], ones_sb[:],
                                     pt[:, i * 256:(i + 1) * 256],
                                     start=(k == 0), stop=(k == n_mm - 1))
                    k += 1
            cx = ps.tile([HD, 256], F32, tag="ps", name="cx")
            k = 0
            for tp, pt in enumerate(pts):
                for i in range(2):
                    nc.tensor.matmul(cx[:], v_sb[2 * tp + i][:],
                                     pt[:, i * 256:(i + 1) * 256],
                                     start=(k == 0), stop=(k == n_mm - 1))
                    k += 1
            rec = tmp.tile([1, 256], F32, tag="rec", name="rec", bufs=2)
            nc.vector.reciprocal(rec[:], den[:])
            bc = tmp.tile([128, 256], F32, tag="bc", name="bc", bufs=2)
            nc.gpsimd.partition_broadcast(bc[:], rec[:])
            ctxn = tmp.tile([HD, 256], F32, tag="ctxn", name="ctxn", bufs=2)
            nc.vector.scalar_tensor_tensor(
                ctxn[:], cx[:], CTXS, bc[:],
                op0=mybir.AluOpType.mult, op1=mybir.AluOpType.mult)
            u, par = divmod(h, 2)
            nc.scalar.activation(ctx_hi[u][:, par, ssl], ctxn[:],
                                 mybir.ActivationFunctionType.Copy)
            nc.vector.tensor_sub(ctx_lo[u][:, par, ssl], ctxn[:],
                                 ctx_hi[u][:, par, ssl])

        descale = 1.0 / (CTXS * WOS)

        def emit_outproj(t8):
            tsl = slice(t8 * 128, (t8 + 1) * 128)
            for n4 in range(4):
                ot = outp.tile([128, 1024], BF, tag="ot", name="ot")
                for sub in range(4):
                    n = 4 * n4 + sub
                    nsl = slice(n * 256, (n + 1) * 256)
                    op = ps.tile([128, 256], F32, tag="ps", name="op")
                    k = 0
                    for u in range(2):
                        for chi, whi in ((ctx_hi, 0), (ctx_lo, 0), (ctx_hi, 1)):
                            nc.tensor.matmul(
                                op[:], chi[u][:, :, tsl],
                                wo_sb[:, whi, 2 * u:2 * u + 2, nsl],
                                start=(k == 0), stop=(k == 5), perf_mode=DR)
                            k += 1
                    nc.gpsimd.tensor_scalar_mul(ot[:, sub * 256:(sub + 1) * 256],
                                                op[:], descale)
                nc.sync.dma_start(
                    out=out.ap()[tsl, n4 * 1024:(n4 + 1) * 1024], in_=ot[:])

        stages = [(h, b) for b in range(4) for h in range(HPC)]
        prev = None
        outq = []
        for hb in stages:
            pts = emit_scores(*hb)
            if vq:
                v_chain(vq.pop(0))
            if prev is not None:
                (ph, pb), ppts = prev
                emit_denctx(ph, pb, ppts)
                if ph == HPC - 1:
                    outq.extend([2 * pb, 2 * pb + 1])
            if outq:
                emit_outproj(outq.pop(0))
            prev = (hb, pts)
        (ph, pb), ppts = prev
        emit_denctx(ph, pb, ppts)
        outq.extend([2 * pb, 2 * pb + 1])
        for t8 in outq:
            emit_outproj(t8)


def _prep_inputs(x, cos, sin, Wq, Wk, Wv, Wo):
    """Host-side shard + hi/lo fp8 quantization. Returns per-core inputs."""
    bf = ml_dtypes.bfloat16
    f8 = ml_dtypes.float8_e4m3

    def hilo(a, s):
        hi = np.asarray(a * s, np.float32).astype(f8)
        lo = (np.asarray(a * s, np.float32) - hi.astype(np.float32)).astype(f8)
        return hi, lo

    x2 = np.asarray(x, np.float32).reshape(S, D)
    xTh = np.ascontiguousarray(x2.T).reshape(NDC, 128, S).transpose(1, 0, 2)
    xh_, xl_ = hilo(np.ascontiguousarray(xTh), XS)

    cosT = np.ascontiguousarray(np.asarray(cos, np.float32).T).astype(bf)
    sinT = np.ascontiguousarray(np.asarray(sin, np.float32).T).astype(bf)

    rmat = np.zeros((HD, HD), np.float32)
    half = HD // 2
    rmat[np.arange(half), np.arange(half) + half] = 1.0
    rmat[np.arange(half) + half, np.arange(half)] = -1.0
    rmat = rmat.astype(bf)

    # diagonal pair mask: keep when t_local (= i*128 + p) <= s_local
    lt = np.arange(128)[:, None]
    ls = np.arange(256)[None, :]
    masks = np.concatenate([(lt + 128 * i <= ls) for i in range(2)], axis=1)
    masks = np.ascontiguousarray(masks).astype(bf)     # [128, 512]

    scale = 1.0 / np.sqrt(np.float32(HD))
    Wq_ = np.asarray(Wq, np.float32) * scale
    Wk_ = np.asarray(Wk, np.float32)
    Wv_ = np.asarray(Wv, np.float32)
    Wo_ = np.asarray(Wo, np.float32)

    def chunked(w):  # [D, m] -> [128, NDC, m]
        m = w.shape[1]
        return np.ascontiguousarray(
            w.reshape(NDC, 128, m).transpose(1, 0, 2))

    in_maps = []
    for r in range(N_CORES):
        wqh_, wql_ = hilo(chunked(Wq_[:, r * QW:(r + 1) * QW]), WQS)
        wkh_, wkl_ = hilo(chunked(Wk_[:, r * HD:(r + 1) * HD]), WKS)
        wvh_, wvl_ = hilo(chunked(Wv_[:, r * HD:(r + 1) * HD]), WVS)
        wo_r = np.ascontiguousarray(
            Wo_[r * QW:(r + 1) * QW, :].reshape(HPC, 128, D)
            .transpose(1, 0, 2))
        woh_, wol_ = hilo(wo_r, WOS)
        in_maps.append({
            "xh": xh_, "xl": xl_, "wqh": wqh_, "wql": wql_,
            "wkh": wkh_, "wkl": wkl_, "wvh": wvh_, "wvl": wvl_,
            "woh": woh_, "wol": wol_,
            "cosT": cosT, "sinT": sinT, "rmat": rmat, "masks": masks,
        })
    return in_maps


def get_nc():
    if "nc" not in _CACHE:
        _CACHE["nc"] = _build()
    return _CACHE["nc"]


def kernel(x, mask, cos, sin, Wq, Wk, Wv, Wo):
    nc = get_nc()
    in_maps = _prep_inputs(x, cos, sin, Wq, Wk, Wv, Wo)
    res = run_bass_kernel_spmd(nc, in_maps, core_ids=list(range(N_CORES)))
    acc = np.zeros((S, D), np.float32)
    for r in range(N_CORES):
        acc += res.results[r]["out"].astype(np.float32)
    return acc[None]


if __name__ == "__main__":
    print("built:", get_nc() is not None)


# revision 14
# speedup vs baseline: 1.0909x; 1.0783x over previous
"""Grouped-query attention, tensor-parallel over heads across 8 TRN2 NeuronCores.

Problem (hardcoded): x[1,1024,4096] @ Wq/Wk/Wv -> RoPE -> causal GQA
(32 q heads, 8 kv groups, head_dim 128) -> out proj Wo -> [1,1024,4096].

Sharding: core r owns q heads 4r..4r+3 and kv group r (Wq/Wk/Wv column
shards, Wo row shard). Each core computes a full [1024,4096] partial of
the output projection; the host sums the 8 partials (the "all-reduce").

Device kernel (per core): the big GEMMs (Q/K/V projections, out-proj)
run in fp8e4 DoubleRow mode, which processes two 128-deep contraction
chunks per instruction at half the cycles/row of bf16.  Precision is
recovered with a 3-term hi/lo split quantization (x_hi@W_hi + x_lo@W_hi
+ x_hi@W_lo), where hi and lo shares one power-of-2 scale so all terms
accumulate in a single PSUM chain; measured end-to-end error matches
bf16.  The attention core (scores, exp, denominators, ctx) stays bf16
with 256-wide s-blocks and causal tile skipping.
"""

import numpy as np
import ml_dtypes

import concourse.bass as bass
import concourse.bacc as bacc
import concourse.mybir as mybir
import concourse.tile as tile
from concourse.bass_utils import run_bass_kernel_spmd

S = 1024          # sequence length
D = 4096          # model dim
H = 32            # query heads (global)
G = 8             # kv groups (global)
HD = 128          # head dim
N_CORES = 8
HPC = H // N_CORES   # 4 query heads per core
QW = HPC * HD        # 512 q-proj cols per core
NDC = D // 128       # 32 contraction chunks
NP = NDC // 2        # 16 DoubleRow chunk pairs
BF = mybir.dt.bfloat16
F8 = mybir.dt.float8e4
F32 = mybir.dt.float32
DR = mybir.MatmulPerfMode.DoubleRow

# quantization scales (powers of 2; hi and lo share the scale so every
# 3-term matmul accumulates in one PSUM chain)
XS = 16.0
WQS = 8192.0        # applied to Wq/sqrt(HD)
WKS = 1024.0
WVS = 1024.0
WOS = 1024.0
CTXS = 16.0
EXP_SHIFT = -6.0    # exp(s - 6): keeps bf16 P comfortably in range

_CACHE = {}


def _build():
    nc = bacc.Bacc("TRN2", target_bir_lowering=False, debug=False,
                   num_devices=N_CORES)

    xh = nc.dram_tensor("xh", [128, NDC, S], F8, kind="ExternalInput")
    xl = nc.dram_tensor("xl", [128, NDC, S], F8, kind="ExternalInput")
    wqh = nc.dram_tensor("wqh", [128, NDC, QW], F8, kind="ExternalInput")
    wql = nc.dram_tensor("wql", [128, NDC, QW], F8, kind="ExternalInput")
    wkh = nc.dram_tensor("wkh", [128, NDC, HD], F8, kind="ExternalInput")
    wkl = nc.dram_tensor("wkl", [128, NDC, HD], F8, kind="ExternalInput")
    wvh = nc.dram_tensor("wvh", [128, NDC, HD], F8, kind="ExternalInput")
    wvl = nc.dram_tensor("wvl", [128, NDC, HD], F8, kind="ExternalInput")
    woh = nc.dram_tensor("woh", [128, HPC, D], F8, kind="ExternalInput")
    wol = nc.dram_tensor("wol", [128, HPC, D], F8, kind="ExternalInput")
    cosT = nc.dram_tensor("cosT", [HD, S], BF, kind="ExternalInput")
    sinT = nc.dram_tensor("sinT", [HD, S], BF, kind="ExternalInput")
    rmat = nc.dram_tensor("rmat", [HD, HD], BF, kind="ExternalInput")
    masks = nc.dram_tensor("masks", [128, 512], BF, kind="ExternalInput")
    out = nc.dram_tensor("out", [S, D], BF, kind="ExternalOutput")

    with tile.TileContext(nc) as tc:
        _emit(tc, nc, xh, xl, wqh, wql, wkh, wkl, wvh, wvl, woh, wol,
              cosT, sinT, rmat, masks, out)
    nc.compile()
    return nc


def _emit(tc, nc, xh, xl, wqh, wql, wkh, wkl, wvh, wvl, woh, wol,
          cosT, sinT, rmat, masks, out):
    import contextlib
    ctx = contextlib.ExitStack()
    with ctx:
        const = ctx.enter_context(tc.tile_pool(name="const", bufs=1))
        work = ctx.enter_context(tc.tile_pool(name="work", bufs=1))
        tmp = ctx.enter_context(tc.tile_pool(name="tmp", bufs=4))
        pt_pool = ctx.enter_context(tc.tile_pool(name="pt", bufs=8))
        outp = ctx.enter_context(tc.tile_pool(name="outp", bufs=3))
        ps = ctx.enter_context(tc.tile_pool(name="ps", bufs=8, space="PSUM"))

        # ---- DMA emission, ordered to pace the chunk-major PE stream ----
        rmat_sb = const.tile([HD, HD], BF, tag="rmat")
        ones_sb = const.tile([128, 1], BF, tag="ones")
        nc.vector.memset(ones_sb[:], 1.0)
        ebias = const.tile([128, 1], F32, tag="ebias")
        nc.vector.memset(ebias[:], EXP_SHIFT)

        wk_sb = const.tile([128, 2, NDC, HD], F8, tag="wk")   # dim1: hi/lo
        nc.sync.dma_start(out=rmat_sb[:], in_=rmat.ap())

        gx4h, gx4l = {}, {}
        gqh, gql = {}, {}
        for c in range(0, NDC, 4):
            gx4h[c] = const.tile([128, 4, S], F8, tag=f"xh{c//4}", name=f"xh{c//4}")
            gx4l[c] = const.tile([128, 4, S], F8, tag=f"xl{c//4}", name=f"xl{c//4}")
        gxh = {c: gx4h[c - c % 4][:, c % 4:c % 4 + 2, :] for c in range(0, NDC, 2)}
        gxl = {c: gx4l[c - c % 4][:, c % 4:c % 4 + 2, :] for c in range(0, NDC, 2)}
        # half-0 of x, wq, and wk interleaved in consumption order
        for c in range(0, NDC, 4):
            nc.sync.dma_start(out=wk_sb[:, 0, c:c + 4, :], in_=wkh.ap()[:, c:c + 4, :])
            g = const.tile([128, 4, QW], F8, tag=f"qh{c//4}", name=f"qh{c//4}")
            nc.sync.dma_start(out=g[:], in_=wqh.ap()[:, c:c + 4, :])
            gqh[c], gqh[c + 2] = g[:, 0:2, :], g[:, 2:4, :]
            nc.sync.dma_start(out=gx4h[c][:, :, 0:512], in_=xh.ap()[:, c:c + 4, 0:512])
            nc.sync.dma_start(out=wk_sb[:, 1, c:c + 4, :], in_=wkl.ap()[:, c:c + 4, :])
            g = const.tile([128, 4, QW], F8, tag=f"ql{c//4}", name=f"ql{c//4}")
            nc.sync.dma_start(out=g[:], in_=wql.ap()[:, c:c + 4, :])
            gql[c], gql[c + 2] = g[:, 0:2, :], g[:, 2:4, :]
            nc.sync.dma_start(out=gx4l[c][:, :, 0:512], in_=xl.ap()[:, c:c + 4, 0:512])
            if c == 0:
                wv_sb = const.tile([128, 2, NDC, HD], F8, tag="wv")
                nc.sync.dma_start(out=wv_sb[:, 0, :, :], in_=wvh.ap())
            if c == 4:
                nc.sync.dma_start(out=wv_sb[:, 1, :, :], in_=wvl.ap())
        cos_sb = const.tile([HD, S], BF, tag="cos")
        nc.sync.dma_start(out=cos_sb[:], in_=cosT.ap())
        sin_sb = const.tile([HD, S], BF, tag="sin")
        nc.sync.dma_start(out=sin_sb[:], in_=sinT.ap())
        # half-1 of x
        for c in range(0, NDC, 4):
            nc.sync.dma_start(out=gx4h[c][:, :, 512:S], in_=xh.ap()[:, c:c + 4, 512:S])
            nc.sync.dma_start(out=gx4l[c][:, :, 512:S], in_=xl.ap()[:, c:c + 4, 512:S])
        mask_sb = const.tile([128, 512], BF, tag="mask")
        nc.sync.dma_start(out=mask_sb[:], in_=masks.ap())
        wo_sb = const.tile([128, 2, HPC, D], F8, tag="wo")    # dim1: hi/lo
        for n in range(2):
            sl = slice(n * 2048, (n + 1) * 2048)
            nc.sync.dma_start(out=wo_sb[:, 0, :, sl], in_=woh.ap()[:, :, sl])
            nc.sync.dma_start(out=wo_sb[:, 1, :, sl], in_=wol.ap()[:, :, sl])

        # persistent activations
        khat = work.tile([HD, S], BF, tag="khat")
        qhat = [work.tile([HD, S], BF, tag=f"qhat{h}", name=f"qhat{h}")
                for h in range(HPC)]
        v_sb = [work.tile([128, HD], BF, tag=f"v{i}", name=f"v{i}")
                for i in range(8)]
        ctx_hi = [work.tile([128, 2, S], F8, tag=f"cth{u}", name=f"cth{u}")
                  for u in range(2)]
        ctx_lo = [work.tile([128, 2, S], F8, tag=f"ctl{u}", name=f"ctl{u}")
                  for u in range(2)]

        # ---- K+Q projections: chunk-major across 5 chains per s-quarter ----
        # Per chunk pair, all five tensors advance their 3-term DoubleRow
        # chains, so the PE stream follows the x/wq DMA arrival order.
        # RoPE for each finished s-half is queued and its PE/Act/DVE work is
        # injected into later quarters' streams (and the v-projection).
        TENS = [("k", khat, lambda c: wk_sb[:, 0, c:c + 2, :],
                 lambda c: wk_sb[:, 1, c:c + 2, :], 1.0 / (XS * WKS))]
        for h in range(HPC):
            hsl = slice(h * HD, (h + 1) * HD)
            TENS.append((f"q{h}", qhat[h],
                         lambda c, s=hsl: gqh[c][:, :, s],
                         lambda c, s=hsl: gql[c][:, :, s], 1.0 / (XS * WQS)))
        raws = {ti: work.tile([HD, S], BF, tag=f"raw{ti}", name=f"raw{ti}")
                for ti in range(5)}

        pend = []   # queued rope-finish closures (one per (tensor, half))

        def inject_rope():
            if pend:
                pend.pop(0)()

        def rope_half(ti, half):
            name, dst, _, _, _ = TENS[ti]
            sl = slice(half * 512, (half + 1) * 512)
            t1 = tmp.tile([HD, 512], BF, tag="rope_t1", name="rope_t1", bufs=2)
            nc.vector.tensor_mul(t1[:], raws[ti][:, sl], cos_sb[:, sl])
            rq = ps.tile([HD, 512], F32, tag="ps", name="rq")
            nc.tensor.matmul(rq[:], rmat_sb[:], raws[ti][:, sl],
                             start=True, stop=True)
            rqs = tmp.tile([HD, 512], BF, tag="rope_rqs", name="rope_rqs", bufs=2)
            nc.scalar.activation(rqs[:], rq[:],
                                 mybir.ActivationFunctionType.Copy)
            t2 = tmp.tile([HD, 512], BF, tag="rope_t2", name="rope_t2", bufs=2)
            nc.vector.tensor_mul(t2[:], rqs[:], sin_sb[:, sl])
            nc.vector.tensor_add(dst[:, sl], t1[:], t2[:])

        def v_terms(vpsum, i, p):
            c = 2 * p
            tsl = slice(i * 128, (i + 1) * 128)
            nc.tensor.matmul(vpsum[:], gxh[c][:, :, tsl], wv_sb[:, 0, c:c + 2, :],
                             start=(p == 0), stop=False,
                             perf_mode=DR, skip_group_check=True)
            nc.tensor.matmul(vpsum[:], gxh[c][:, :, tsl], wv_sb[:, 1, c:c + 2, :],
                             start=False, stop=False,
                             perf_mode=DR, skip_group_check=True)
            nc.tensor.matmul(vpsum[:], gxl[c][:, :, tsl], wv_sb[:, 0, c:c + 2, :],
                             start=False, stop=(p == NP - 1),
                             perf_mode=DR, skip_group_check=True)

        for half in (0, 1):
            hoff = half * 512
            chains = [ps.tile([128, 512], F32, tag="ps", name=f"ch{ti}")
                      for ti in range(5)]
            for p in range(NP):
                c = 2 * p
                for term in range(3):
                    for ti, (_, _, whi, wlo, _) in enumerate(TENS):
                        pp = chains[ti]
                        w = whi(c) if term != 1 else wlo(c)
                        for q in range(2):
                            ssl = slice(hoff + q * 256, hoff + (q + 1) * 256)
                            osl = slice(q * 256, (q + 1) * 256)
                            xop = gxl[c] if term == 2 else gxh[c]
                            nc.tensor.matmul(
                                pp[:, osl], w, xop[:, :, ssl],
                                start=(p == 0 and term == 0 and q == 0),
                                stop=(p == NP - 1 and term == 2 and q == 1),
                                perf_mode=DR, skip_group_check=True)
                if p in (4, 9, 14):
                    inject_rope()
            for ti, (_, _, _, _, descale) in enumerate(TENS):
                nc.scalar.activation(raws[ti][:, hoff:hoff + 512], chains[ti][:],
                                     mybir.ActivationFunctionType.Copy,
                                     scale=descale)
            for ti in range(5):
                pend.append(lambda t=ti, hf=half: rope_half(t, hf))

        # ---- V projection: emitted as PE filler inside early attention ----
        def v_chain(i):
            tsl = slice(i * 128, (i + 1) * 128)
            vp = ps.tile([128, HD], F32, tag="ps", name="vp")
            for p in range(NP):
                c = 2 * p
                nc.tensor.matmul(vp[:], gxh[c][:, :, tsl], wv_sb[:, 0, c:c + 2, :],
                                 start=(p == 0), stop=False, perf_mode=DR)
            for p in range(NP):
                c = 2 * p
                nc.tensor.matmul(vp[:], gxh[c][:, :, tsl], wv_sb[:, 1, c:c + 2, :],
                                 start=False, stop=False, perf_mode=DR)
            for p in range(NP):
                c = 2 * p
                nc.tensor.matmul(vp[:], gxl[c][:, :, tsl], wv_sb[:, 0, c:c + 2, :],
                                 start=False, stop=(p == NP - 1), perf_mode=DR)
            nc.scalar.activation(v_sb[i][:], vp[:],
                                 mybir.ActivationFunctionType.Copy,
                                 scale=1.0 / (XS * WVS))
            inject_rope()

        v_chain(0)
        v_chain(1)
        vq = list(range(2, 8))
        while pend and len(vq) > 4:
            v_chain(vq.pop(0))
        while pend:
            inject_rope()

        # ---- attention + out-proj, software-pipelined ----
        def emit_scores(h, b):
            ssl = slice(b * 256, (b + 1) * 256)
            pts = []
            for tp in range(b + 1):
                st = ps.tile([128, 512], F32, tag="ps", name="st")
                for i in range(2):
                    t0 = (2 * tp + i) * 128
                    nc.tensor.matmul(st[:, i * 256:(i + 1) * 256],
                                     khat[:, t0:t0 + 128], qhat[h][:, ssl],
                                     start=True, stop=True)
                pt = pt_pool.tile([128, 512], BF, tag="pt", name="pt")
                nc.scalar.activation(pt[:], st[:],
                                     mybir.ActivationFunctionType.Exp,
                                     bias=ebias[:])
                if tp == b:
                    nc.vector.tensor_mul(pt[:], pt[:], mask_sb[:])
                pts.append(pt)
            return pts

        def emit_denctx(h, b, pts):
            ssl = slice(b * 256, (b + 1) * 256)
            den = ps.tile([1, 256], F32, tag="ps", name="den")
            n_mm = 2 * (b + 1)
            k = 0
            for pt in pts:
                for i in range(2):
                    nc.tensor.matmul(den[:], ones_sb[:],
                                     pt[:, i * 256:(i + 1) * 256],
                                     start=(k == 0), stop=(k == n_mm - 1))
                    k += 1
            cx = ps.tile([HD, 256], F32, tag="ps", name="cx")
            k = 0
            for tp, pt in enumerate(pts):
                for i in range(2):
                    nc.tensor.matmul(cx[:], v_sb[2 * tp + i][:],
                                     pt[:, i * 256:(i + 1) * 256],
                                     start=(k == 0), stop=(k == n_mm - 1))
                    k += 1
            rec = tmp.tile([1, 256], F32, tag="rec", name="rec", bufs=2)
            nc.vector.reciprocal(rec[:], den[:])
            bc = tmp.tile([128, 256], F32, tag="bc", name="bc", bufs=2)
            nc.gpsimd.partition_broadcast(bc[:], rec[:])
            ctxn = tmp.tile([HD, 256], F32, tag="ctxn", name="ctxn", bufs=2)
            nc.vector.scalar_tensor_tensor(
                ctxn[:], cx[:], CTXS, bc[:],
                op0=mybir.AluOpType.mult, op1=mybir.AluOpType.mult)
            u, par = divmod(h, 2)
            nc.scalar.activation(ctx_hi[u][:, par, ssl], ctxn[:],
                                 mybir.ActivationFunctionType.Copy)
            nc.vector.tensor_sub(ctx_lo[u][:, par, ssl], ctxn[:],
                                 ctx_hi[u][:, par, ssl])

        descale = 1.0 / (CTXS * WOS)

        def emit_outproj(t8):
            tsl = slice(t8 * 128, (t8 + 1) * 128)
            for n4 in range(4):
                ot = outp.tile([128, 1024], BF, tag="ot", name="ot")
                for sub in range(4):
                    n = 4 * n4 + sub
                    nsl = slice(n * 256, (n + 1) * 256)
                    op = ps.tile([128, 256], F32, tag="ps", name="op")
                    k = 0
                    for u in range(2):
                        for chi, whi in ((ctx_hi, 0), (ctx_lo, 0), (ctx_hi, 1)):
                            nc.tensor.matmul(
                                op[:], chi[u][:, :, tsl],
                                wo_sb[:, whi, 2 * u:2 * u + 2, nsl],
                                start=(k == 0), stop=(k == 5), perf_mode=DR)
                            k += 1
                    eng = nc.vector if (sub % 2 == 0) else nc.gpsimd
                    eng.tensor_scalar_mul(ot[:, sub * 256:(sub + 1) * 256],
                                          op[:], descale)
                nc.sync.dma_start(
                    out=out.ap()[tsl, n4 * 1024:(n4 + 1) * 1024], in_=ot[:])

        stages = [(h, b) for b in range(4) for h in range(HPC)]
        prev = None
        outq = []
        for hb in stages:
            pts = emit_scores(*hb)
            if vq:
                v_chain(vq.pop(0))
            if prev is not None:
                (ph, pb), ppts = prev
                emit_denctx(ph, pb, ppts)
                if ph == HPC - 1:
                    outq.extend([2 * pb, 2 * pb + 1])
            if outq:
                emit_outproj(outq.pop(0))
            prev = (hb, pts)
        (ph, pb), ppts = prev
        emit_denctx(ph, pb, ppts)
        outq.extend([2 * pb, 2 * pb + 1])
        for t8 in outq:
            emit_outproj(t8)


def _prep_inputs(x, cos, sin, Wq, Wk, Wv, Wo):
    """Host-side shard + hi/lo fp8 quantization. Returns per-core inputs."""
    bf = ml_dtypes.bfloat16
    f8 = ml_dtypes.float8_e4m3

    def hilo(a, s):
        hi = np.asarray(a * s, np.float32).astype(f8)
        lo = (np.asarray(a * s, np.float32) - hi.astype(np.float32)).astype(f8)
        return hi, lo

    x2 = np.asarray(x, np.float32).reshape(S, D)
    xTh = np.ascontiguousarray(x2.T).reshape(NDC, 128, S).transpose(1, 0, 2)
    xh_, xl_ = hilo(np.ascontiguousarray(xTh), XS)

    cosT = np.ascontiguousarray(np.asarray(cos, np.float32).T).astype(bf)
    sinT = np.ascontiguousarray(np.asarray(sin, np.float32).T).astype(bf)

    rmat = np.zeros((HD, HD), np.float32)
    half = HD // 2
    rmat[np.arange(half), np.arange(half) + half] = 1.0
    rmat[np.arange(half) + half, np.arange(half)] = -1.0
    rmat = rmat.astype(bf)

    # diagonal pair mask: keep when t_local (= i*128 + p) <= s_local
    lt = np.arange(128)[:, None]
    ls = np.arange(256)[None, :]
    masks = np.concatenate([(lt + 128 * i <= ls) for i in range(2)], axis=1)
    masks = np.ascontiguousarray(masks).astype(bf)     # [128, 512]

    scale = 1.0 / np.sqrt(np.float32(HD))
    Wq_ = np.asarray(Wq, np.float32) * scale
    Wk_ = np.asarray(Wk, np.float32)
    Wv_ = np.asarray(Wv, np.float32)
    Wo_ = np.asarray(Wo, np.float32)

    def chunked(w):  # [D, m] -> [128, NDC, m]
        m = w.shape[1]
        return np.ascontiguousarray(
            w.reshape(NDC, 128, m).transpose(1, 0, 2))

    in_maps = []
    for r in range(N_CORES):
        wqh_, wql_ = hilo(chunked(Wq_[:, r * QW:(r + 1) * QW]), WQS)
        wkh_, wkl_ = hilo(chunked(Wk_[:, r * HD:(r + 1) * HD]), WKS)
        wvh_, wvl_ = hilo(chunked(Wv_[:, r * HD:(r + 1) * HD]), WVS)
        wo_r = np.ascontiguousarray(
            Wo_[r * QW:(r + 1) * QW, :].reshape(HPC, 128, D)
            .transpose(1, 0, 2))
        woh_, wol_ = hilo(wo_r, WOS)
        in_maps.append({
            "xh": xh_, "xl": xl_, "wqh": wqh_, "wql": wql_,
            "wkh": wkh_, "wkl": wkl_, "wvh": wvh_, "wvl": wvl_,
            "woh": woh_, "wol": wol_,
            "cosT": cosT, "sinT": sinT, "rmat": rmat, "masks": masks,
        })
    return in_maps


def get_nc():
    if "nc" not in _CACHE:
        _CACHE["nc"] = _build()
    return _CACHE["nc"]


def kernel(x, mask, cos, sin, Wq, Wk, Wv, Wo):
    nc = get_nc()
    in_maps = _prep_inputs(x, cos, sin, Wq, Wk, Wv, Wo)
    res = run_bass_kernel_spmd(nc, in_maps, core_ids=list(range(N_CORES)))
    acc = np.zeros((S, D), np.float32)
    for r in range(N_CORES):
        acc += res.results[r]["out"].astype(np.float32)
    return acc[None]


if __name__ == "__main__":
    print("built:", get_nc() is not None)


# revision 15
# speedup vs baseline: 1.1001x; 1.0084x over previous
"""Grouped-query attention, tensor-parallel over heads across 8 TRN2 NeuronCores.

Problem (hardcoded): x[1,1024,4096] @ Wq/Wk/Wv -> RoPE -> causal GQA
(32 q heads, 8 kv groups, head_dim 128) -> out proj Wo -> [1,1024,4096].

Sharding: core r owns q heads 4r..4r+3 and kv group r (Wq/Wk/Wv column
shards, Wo row shard). Each core computes a full [1024,4096] partial of
the output projection; the host sums the 8 partials (the "all-reduce").

Device kernel (per core): the big GEMMs (Q/K/V projections, out-proj)
run in fp8e4 DoubleRow mode, which processes two 128-deep contraction
chunks per instruction at half the cycles/row of bf16.  Precision is
recovered with a 3-term hi/lo split quantization (x_hi@W_hi + x_lo@W_hi
+ x_hi@W_lo), where hi and lo shares one power-of-2 scale so all terms
accumulate in a single PSUM chain; measured end-to-end error matches
bf16.  The attention core (scores, exp, denominators, ctx) stays bf16
with 256-wide s-blocks and causal tile skipping.
"""

import numpy as np
import ml_dtypes

import concourse.bass as bass
import concourse.bacc as bacc
import concourse.mybir as mybir
import concourse.tile as tile
from concourse.bass_utils import run_bass_kernel_spmd

S = 1024          # sequence length
D = 4096          # model dim
H = 32            # query heads (global)
G = 8             # kv groups (global)
HD = 128          # head dim
N_CORES = 8
HPC = H // N_CORES   # 4 query heads per core
QW = HPC * HD        # 512 q-proj cols per core
NDC = D // 128       # 32 contraction chunks
NP = NDC // 2        # 16 DoubleRow chunk pairs
BF = mybir.dt.bfloat16
F8 = mybir.dt.float8e4
F32 = mybir.dt.float32
DR = mybir.MatmulPerfMode.DoubleRow

# quantization scales (powers of 2; hi and lo share the scale so every
# 3-term matmul accumulates in one PSUM chain)
XS = 16.0
WQS = 8192.0        # applied to Wq/sqrt(HD)
WKS = 1024.0
WVS = 1024.0
WOS = 1024.0
CTXS = 16.0
EXP_SHIFT = -6.0    # exp(s - 6): keeps bf16 P comfortably in range

_CACHE = {}


def _build():
    nc = bacc.Bacc("TRN2", target_bir_lowering=False, debug=False,
                   num_devices=N_CORES)

    xh = nc.dram_tensor("xh", [128, NDC, S], F8, kind="ExternalInput")
    xl = nc.dram_tensor("xl", [128, NDC, S], F8, kind="ExternalInput")
    wqh = nc.dram_tensor("wqh", [128, NDC, QW], F8, kind="ExternalInput")
    wql = nc.dram_tensor("wql", [128, NDC, QW], F8, kind="ExternalInput")
    wkh = nc.dram_tensor("wkh", [128, NDC, HD], F8, kind="ExternalInput")
    wkl = nc.dram_tensor("wkl", [128, NDC, HD], F8, kind="ExternalInput")
    wvh = nc.dram_tensor("wvh", [128, NDC, HD], F8, kind="ExternalInput")
    wvl = nc.dram_tensor("wvl", [128, NDC, HD], F8, kind="ExternalInput")
    woh = nc.dram_tensor("woh", [128, HPC, D], F8, kind="ExternalInput")
    wol = nc.dram_tensor("wol", [128, HPC, D], F8, kind="ExternalInput")
    cosT = nc.dram_tensor("cosT", [HD, S], BF, kind="ExternalInput")
    sinT = nc.dram_tensor("sinT", [HD, S], BF, kind="ExternalInput")
    rmat = nc.dram_tensor("rmat", [HD, HD], BF, kind="ExternalInput")
    masks = nc.dram_tensor("masks", [128, 512], BF, kind="ExternalInput")
    out = nc.dram_tensor("out", [S, D], BF, kind="ExternalOutput")

    with tile.TileContext(nc) as tc:
        _emit(tc, nc, xh, xl, wqh, wql, wkh, wkl, wvh, wvl, woh, wol,
              cosT, sinT, rmat, masks, out)
    nc.compile()
    return nc


def _emit(tc, nc, xh, xl, wqh, wql, wkh, wkl, wvh, wvl, woh, wol,
          cosT, sinT, rmat, masks, out):
    import contextlib
    ctx = contextlib.ExitStack()
    with ctx:
        const = ctx.enter_context(tc.tile_pool(name="const", bufs=1))
        work = ctx.enter_context(tc.tile_pool(name="work", bufs=1))
        tmp = ctx.enter_context(tc.tile_pool(name="tmp", bufs=4))
        pt_pool = ctx.enter_context(tc.tile_pool(name="pt", bufs=8))
        outp = ctx.enter_context(tc.tile_pool(name="outp", bufs=3))
        ps = ctx.enter_context(tc.tile_pool(name="ps", bufs=8, space="PSUM"))

        # ---- DMA emission, ordered to pace the chunk-major PE stream ----
        rmat_sb = const.tile([HD, HD], BF, tag="rmat")
        ones_sb = const.tile([128, 1], BF, tag="ones")
        nc.vector.memset(ones_sb[:], 1.0)
        ebias = const.tile([128, 1], F32, tag="ebias")
        nc.vector.memset(ebias[:], EXP_SHIFT)

        wk_sb = const.tile([128, 2, NDC, HD], F8, tag="wk")   # dim1: hi/lo
        nc.sync.dma_start(out=rmat_sb[:], in_=rmat.ap())

        gx4h, gx4l = {}, {}
        gqh, gql = {}, {}
        for c in range(0, NDC, 4):
            gx4h[c] = const.tile([128, 4, S], F8, tag=f"xh{c//4}", name=f"xh{c//4}")
            gx4l[c] = const.tile([128, 4, S], F8, tag=f"xl{c//4}", name=f"xl{c//4}")
        gxh = {c: gx4h[c - c % 4][:, c % 4:c % 4 + 2, :] for c in range(0, NDC, 2)}
        gxl = {c: gx4l[c - c % 4][:, c % 4:c % 4 + 2, :] for c in range(0, NDC, 2)}
        # half-0 of x, wq, and wk interleaved in consumption order
        for c in range(0, NDC, 4):
            nc.sync.dma_start(out=wk_sb[:, 0, c:c + 4, :], in_=wkh.ap()[:, c:c + 4, :])
            g = const.tile([128, 4, QW], F8, tag=f"qh{c//4}", name=f"qh{c//4}")
            nc.sync.dma_start(out=g[:], in_=wqh.ap()[:, c:c + 4, :])
            gqh[c], gqh[c + 2] = g[:, 0:2, :], g[:, 2:4, :]
            nc.sync.dma_start(out=gx4h[c][:, :, 0:512], in_=xh.ap()[:, c:c + 4, 0:512])
            nc.sync.dma_start(out=wk_sb[:, 1, c:c + 4, :], in_=wkl.ap()[:, c:c + 4, :])
            g = const.tile([128, 4, QW], F8, tag=f"ql{c//4}", name=f"ql{c//4}")
            nc.sync.dma_start(out=g[:], in_=wql.ap()[:, c:c + 4, :])
            gql[c], gql[c + 2] = g[:, 0:2, :], g[:, 2:4, :]
            nc.sync.dma_start(out=gx4l[c][:, :, 0:512], in_=xl.ap()[:, c:c + 4, 0:512])
        cos_sb = const.tile([HD, S], BF, tag="cos")
        nc.sync.dma_start(out=cos_sb[:], in_=cosT.ap())
        sin_sb = const.tile([HD, S], BF, tag="sin")
        nc.sync.dma_start(out=sin_sb[:], in_=sinT.ap())
        # half-1 of x
        for c in range(0, NDC, 4):
            nc.sync.dma_start(out=gx4h[c][:, :, 512:S], in_=xh.ap()[:, c:c + 4, 512:S])
            nc.sync.dma_start(out=gx4l[c][:, :, 512:S], in_=xl.ap()[:, c:c + 4, 512:S])
        wv_sb = const.tile([128, 2, NDC, HD], F8, tag="wv")
        nc.sync.dma_start(out=wv_sb[:, 0, :, :], in_=wvh.ap())
        nc.sync.dma_start(out=wv_sb[:, 1, :, :], in_=wvl.ap())
        mask_sb = const.tile([128, 512], BF, tag="mask")
        nc.sync.dma_start(out=mask_sb[:], in_=masks.ap())
        wo_sb = const.tile([128, 2, HPC, D], F8, tag="wo")    # dim1: hi/lo
        for n in range(2):
            sl = slice(n * 2048, (n + 1) * 2048)
            nc.sync.dma_start(out=wo_sb[:, 0, :, sl], in_=woh.ap()[:, :, sl])
            nc.sync.dma_start(out=wo_sb[:, 1, :, sl], in_=wol.ap()[:, :, sl])

        # persistent activations
        khat = work.tile([HD, S], BF, tag="khat")
        qhat = [work.tile([HD, S], BF, tag=f"qhat{h}", name=f"qhat{h}")
                for h in range(HPC)]
        v_sb = [work.tile([128, HD], BF, tag=f"v{i}", name=f"v{i}")
                for i in range(8)]
        ctx_hi = [work.tile([128, 2, S], F8, tag=f"cth{u}", name=f"cth{u}")
                  for u in range(2)]
        ctx_lo = [work.tile([128, 2, S], F8, tag=f"ctl{u}", name=f"ctl{u}")
                  for u in range(2)]

        # ---- K+Q projections: chunk-major across 5 chains per s-quarter ----
        # Per chunk pair, all five tensors advance their 3-term DoubleRow
        # chains, so the PE stream follows the x/wq DMA arrival order.
        # RoPE for each finished s-half is queued and its PE/Act/DVE work is
        # injected into later quarters' streams (and the v-projection).
        TENS = [("k", khat, lambda c: wk_sb[:, 0, c:c + 2, :],
                 lambda c: wk_sb[:, 1, c:c + 2, :], 1.0 / (XS * WKS))]
        for h in range(HPC):
            hsl = slice(h * HD, (h + 1) * HD)
            TENS.append((f"q{h}", qhat[h],
                         lambda c, s=hsl: gqh[c][:, :, s],
                         lambda c, s=hsl: gql[c][:, :, s], 1.0 / (XS * WQS)))
        raws = {ti: work.tile([HD, S], BF, tag=f"raw{ti}", name=f"raw{ti}")
                for ti in range(5)}

        pend = []   # queued rope-finish closures (one per (tensor, half))

        def inject_rope():
            if pend:
                pend.pop(0)()

        def rope_half(ti, half):
            name, dst, _, _, _ = TENS[ti]
            sl = slice(half * 512, (half + 1) * 512)
            t1 = tmp.tile([HD, 512], BF, tag="rope_t1", name="rope_t1", bufs=2)
            nc.vector.tensor_mul(t1[:], raws[ti][:, sl], cos_sb[:, sl])
            rq = ps.tile([HD, 512], F32, tag="ps", name="rq")
            nc.tensor.matmul(rq[:], rmat_sb[:], raws[ti][:, sl],
                             start=True, stop=True)
            rqs = tmp.tile([HD, 512], BF, tag="rope_rqs", name="rope_rqs", bufs=2)
            nc.scalar.activation(rqs[:], rq[:],
                                 mybir.ActivationFunctionType.Copy)
            t2 = tmp.tile([HD, 512], BF, tag="rope_t2", name="rope_t2", bufs=2)
            nc.vector.tensor_mul(t2[:], rqs[:], sin_sb[:, sl])
            nc.vector.tensor_add(dst[:, sl], t1[:], t2[:])

        def v_terms(vpsum, i, p):
            c = 2 * p
            tsl = slice(i * 128, (i + 1) * 128)
            nc.tensor.matmul(vpsum[:], gxh[c][:, :, tsl], wv_sb[:, 0, c:c + 2, :],
                             start=(p == 0), stop=False,
                             perf_mode=DR, skip_group_check=True)
            nc.tensor.matmul(vpsum[:], gxh[c][:, :, tsl], wv_sb[:, 1, c:c + 2, :],
                             start=False, stop=False,
                             perf_mode=DR, skip_group_check=True)
            nc.tensor.matmul(vpsum[:], gxl[c][:, :, tsl], wv_sb[:, 0, c:c + 2, :],
                             start=False, stop=(p == NP - 1),
                             perf_mode=DR, skip_group_check=True)

        for half in (0, 1):
            hoff = half * 512
            chains = [ps.tile([128, 512], F32, tag="ps", name=f"ch{ti}")
                      for ti in range(5)]
            for p in range(NP):
                c = 2 * p
                for term in range(3):
                    for ti, (_, _, whi, wlo, _) in enumerate(TENS):
                        pp = chains[ti]
                        w = whi(c) if term != 1 else wlo(c)
                        for q in range(2):
                            ssl = slice(hoff + q * 256, hoff + (q + 1) * 256)
                            osl = slice(q * 256, (q + 1) * 256)
                            xop = gxl[c] if term == 2 else gxh[c]
                            nc.tensor.matmul(
                                pp[:, osl], w, xop[:, :, ssl],
                                start=(p == 0 and term == 0 and q == 0),
                                stop=(p == NP - 1 and term == 2 and q == 1),
                                perf_mode=DR, skip_group_check=True)
                if p in (4, 9, 14):
                    inject_rope()
            for ti, (_, _, _, _, descale) in enumerate(TENS):
                nc.scalar.activation(raws[ti][:, hoff:hoff + 512], chains[ti][:],
                                     mybir.ActivationFunctionType.Copy,
                                     scale=descale)
            for ti in range(5):
                pend.append(lambda t=ti, hf=half: rope_half(t, hf))

        # ---- V projection: emitted as PE filler inside early attention ----
        def v_chain(i):
            tsl = slice(i * 128, (i + 1) * 128)
            vp = ps.tile([128, HD], F32, tag="ps", name="vp")
            for p in range(NP):
                c = 2 * p
                nc.tensor.matmul(vp[:], gxh[c][:, :, tsl], wv_sb[:, 0, c:c + 2, :],
                                 start=(p == 0), stop=False, perf_mode=DR)
            for p in range(NP):
                c = 2 * p
                nc.tensor.matmul(vp[:], gxh[c][:, :, tsl], wv_sb[:, 1, c:c + 2, :],
                                 start=False, stop=False, perf_mode=DR)
            for p in range(NP):
                c = 2 * p
                nc.tensor.matmul(vp[:], gxl[c][:, :, tsl], wv_sb[:, 0, c:c + 2, :],
                                 start=False, stop=(p == NP - 1), perf_mode=DR)
            nc.scalar.activation(v_sb[i][:], vp[:],
                                 mybir.ActivationFunctionType.Copy,
                                 scale=1.0 / (XS * WVS))
            inject_rope()

        v_chain(0)
        v_chain(1)
        vq = list(range(2, 8))
        while pend and len(vq) > 4:
            v_chain(vq.pop(0))
        while pend:
            inject_rope()

        # ---- attention + out-proj, software-pipelined ----
        def emit_scores(h, b):
            ssl = slice(b * 256, (b + 1) * 256)
            pts = []
            for tp in range(b + 1):
                st = ps.tile([128, 512], F32, tag="ps", name="st")
                for i in range(2):
                    t0 = (2 * tp + i) * 128
                    nc.tensor.matmul(st[:, i * 256:(i + 1) * 256],
                                     khat[:, t0:t0 + 128], qhat[h][:, ssl],
                                     start=True, stop=True)
                pt = pt_pool.tile([128, 512], BF, tag="pt", name="pt")
                nc.scalar.activation(pt[:], st[:],
                                     mybir.ActivationFunctionType.Exp,
                                     bias=ebias[:])
                if tp == b:
                    nc.vector.tensor_mul(pt[:], pt[:], mask_sb[:])
                pts.append(pt)
            return pts

        def emit_denctx(h, b, pts):
            ssl = slice(b * 256, (b + 1) * 256)
            den = ps.tile([1, 256], F32, tag="ps", name="den")
            n_mm = 2 * (b + 1)
            k = 0
            for pt in pts:
                for i in range(2):
                    nc.tensor.matmul(den[:], ones_sb[:],
                                     pt[:, i * 256:(i + 1) * 256],
                                     start=(k == 0), stop=(k == n_mm - 1))
                    k += 1
            cx = ps.tile([HD, 256], F32, tag="ps", name="cx")
            k = 0
            for tp, pt in enumerate(pts):
                for i in range(2):
                    nc.tensor.matmul(cx[:], v_sb[2 * tp + i][:],
                                     pt[:, i * 256:(i + 1) * 256],
                                     start=(k == 0), stop=(k == n_mm - 1))
                    k += 1
            rec = tmp.tile([1, 256], F32, tag="rec", name="rec", bufs=2)
            nc.vector.reciprocal(rec[:], den[:])
            bc = tmp.tile([128, 256], F32, tag="bc", name="bc", bufs=2)
            nc.gpsimd.partition_broadcast(bc[:], rec[:])
            ctxn = tmp.tile([HD, 256], F32, tag="ctxn", name="ctxn", bufs=2)
            nc.vector.scalar_tensor_tensor(
                ctxn[:], cx[:], CTXS, bc[:],
                op0=mybir.AluOpType.mult, op1=mybir.AluOpType.mult)
            u, par = divmod(h, 2)
            nc.scalar.activation(ctx_hi[u][:, par, ssl], ctxn[:],
                                 mybir.ActivationFunctionType.Copy)
            nc.vector.tensor_sub(ctx_lo[u][:, par, ssl], ctxn[:],
                                 ctx_hi[u][:, par, ssl])

        descale = 1.0 / (CTXS * WOS)

        def emit_outproj(t8):
            tsl = slice(t8 * 128, (t8 + 1) * 128)
            for n4 in range(4):
                ot = outp.tile([128, 1024], BF, tag="ot", name="ot")
                for sub in range(4):
                    n = 4 * n4 + sub
                    nsl = slice(n * 256, (n + 1) * 256)
                    op = ps.tile([128, 256], F32, tag="ps", name="op")
                    k = 0
                    for u in range(2):
                        for chi, whi in ((ctx_hi, 0), (ctx_lo, 0), (ctx_hi, 1)):
                            nc.tensor.matmul(
                                op[:], chi[u][:, :, tsl],
                                wo_sb[:, whi, 2 * u:2 * u + 2, nsl],
                                start=(k == 0), stop=(k == 5), perf_mode=DR)
                            k += 1
                    eng = nc.vector if (sub % 2 == 0) else nc.gpsimd
                    eng.tensor_scalar_mul(ot[:, sub * 256:(sub + 1) * 256],
                                          op[:], descale)
                nc.sync.dma_start(
                    out=out.ap()[tsl, n4 * 1024:(n4 + 1) * 1024], in_=ot[:])

        stages = [(h, b) for b in range(4) for h in range(HPC)]
        prev = None
        outq = []
        for hb in stages:
            pts = emit_scores(*hb)
            if vq:
                v_chain(vq.pop(0))
            if prev is not None:
                (ph, pb), ppts = prev
                emit_denctx(ph, pb, ppts)
                if ph == HPC - 1:
                    outq.extend([2 * pb, 2 * pb + 1])
            if outq:
                emit_outproj(outq.pop(0))
            prev = (hb, pts)
        (ph, pb), ppts = prev
        emit_denctx(ph, pb, ppts)
        outq.extend([2 * pb, 2 * pb + 1])
        for t8 in outq:
            emit_outproj(t8)


def _prep_inputs(x, cos, sin, Wq, Wk, Wv, Wo):
    """Host-side shard + hi/lo fp8 quantization. Returns per-core inputs."""
    bf = ml_dtypes.bfloat16
    f8 = ml_dtypes.float8_e4m3

    def hilo(a, s):
        hi = np.asarray(a * s, np.float32).astype(f8)
        lo = (np.asarray(a * s, np.float32) - hi.astype(np.float32)).astype(f8)
        return hi, lo

    x2 = np.asarray(x, np.float32).reshape(S, D)
    xTh = np.ascontiguousarray(x2.T).reshape(NDC, 128, S).transpose(1, 0, 2)
    xh_, xl_ = hilo(np.ascontiguousarray(xTh), XS)

    cosT = np.ascontiguousarray(np.asarray(cos, np.float32).T).astype(bf)
    sinT = np.ascontiguousarray(np.asarray(sin, np.float32).T).astype(bf)

    rmat = np.zeros((HD, HD), np.float32)
    half = HD // 2
    rmat[np.arange(half), np.arange(half) + half] = 1.0
    rmat[np.arange(half) + half, np.arange(half)] = -1.0
    rmat = rmat.astype(bf)

    # diagonal pair mask: keep when t_local (= i*128 + p) <= s_local
    lt = np.arange(128)[:, None]
    ls = np.arange(256)[None, :]
    masks = np.concatenate([(lt + 128 * i <= ls) for i in range(2)], axis=1)
    masks = np.ascontiguousarray(masks).astype(bf)     # [128, 512]

    scale = 1.0 / np.sqrt(np.float32(HD))
    Wq_ = np.asarray(Wq, np.float32) * scale
    Wk_ = np.asarray(Wk, np.float32)
    Wv_ = np.asarray(Wv, np.float32)
    Wo_ = np.asarray(Wo, np.float32)

    def chunked(w):  # [D, m] -> [128, NDC, m]
        m = w.shape[1]
        return np.ascontiguousarray(
            w.reshape(NDC, 128, m).transpose(1, 0, 2))

    in_maps = []
    for r in range(N_CORES):
        wqh_, wql_ = hilo(chunked(Wq_[:, r * QW:(r + 1) * QW]), WQS)
        wkh_, wkl_ = hilo(chunked(Wk_[:, r * HD:(r + 1) * HD]), WKS)
        wvh_, wvl_ = hilo(chunked(Wv_[:, r * HD:(r + 1) * HD]), WVS)
        wo_r = np.ascontiguousarray(
            Wo_[r * QW:(r + 1) * QW, :].reshape(HPC, 128, D)
            .transpose(1, 0, 2))
        woh_, wol_ = hilo(wo_r, WOS)
        in_maps.append({
            "xh": xh_, "xl": xl_, "wqh": wqh_, "wql": wql_,
            "wkh": wkh_, "wkl": wkl_, "wvh": wvh_, "wvl": wvl_,
            "woh": woh_, "wol": wol_,
            "cosT": cosT, "sinT": sinT, "rmat": rmat, "masks": masks,
        })
    return in_maps


def get_nc():
    if "nc" not in _CACHE:
        _CACHE["nc"] = _build()
    return _CACHE["nc"]


def kernel(x, mask, cos, sin, Wq, Wk, Wv, Wo):
    nc = get_nc()
    in_maps = _prep_inputs(x, cos, sin, Wq, Wk, Wv, Wo)
    res = run_bass_kernel_spmd(nc, in_maps, core_ids=list(range(N_CORES)))
    acc = np.zeros((S, D), np.float32)
    for r in range(N_CORES):
        acc += res.results[r]["out"].astype(np.float32)
    return acc[None]


if __name__ == "__main__":
    print("built:", get_nc() is not None)


# revision 18
# speedup vs baseline: 1.1301x; 1.0273x over previous
"""Grouped-query attention, tensor-parallel over heads across 8 TRN2 NeuronCores.

Problem (hardcoded): x[1,1024,4096] @ Wq/Wk/Wv -> RoPE -> causal GQA
(32 q heads, 8 kv groups, head_dim 128) -> out proj Wo -> [1,1024,4096].

Sharding: core r owns q heads 4r..4r+3 and kv group r (Wq/Wk/Wv column
shards, Wo row shard). Each core computes a full [1024,4096] partial of
the output projection; the host sums the 8 partials (the "all-reduce").

Device kernel (per core): the big GEMMs (Q/K/V projections, out-proj)
run in fp8e4 DoubleRow mode, which processes two 128-deep contraction
chunks per instruction at half the cycles/row of bf16.  Precision is
recovered with a 3-term hi/lo split quantization (x_hi@W_hi + x_lo@W_hi
+ x_hi@W_lo), where hi and lo shares one power-of-2 scale so all terms
accumulate in a single PSUM chain; measured end-to-end error matches
bf16.  The attention core (scores, exp, denominators, ctx) stays bf16
with 256-wide s-blocks and causal tile skipping.
"""

import numpy as np
import ml_dtypes

import concourse.bass as bass
import concourse.bacc as bacc
import concourse.mybir as mybir
import concourse.tile as tile
from concourse.bass_utils import run_bass_kernel_spmd

S = 1024          # sequence length
D = 4096          # model dim
H = 32            # query heads (global)
G = 8             # kv groups (global)
HD = 128          # head dim
N_CORES = 8
HPC = H // N_CORES   # 4 query heads per core
QW = HPC * HD        # 512 q-proj cols per core
NDC = D // 128       # 32 contraction chunks
NP = NDC // 2        # 16 DoubleRow chunk pairs
BF = mybir.dt.bfloat16
F8 = mybir.dt.float8e4
F32 = mybir.dt.float32
DR = mybir.MatmulPerfMode.DoubleRow

# quantization scales (powers of 2; hi and lo share the scale so every
# 3-term matmul accumulates in one PSUM chain)
XS = 16.0
WQS = 8192.0        # applied to Wq/sqrt(HD)
WKS = 1024.0
WVS = 1024.0
WOS = 1024.0
CTXS = 16.0
EXP_SHIFT = -6.0    # exp(s - 6): keeps bf16 P comfortably in range

_CACHE = {}


def _build():
    nc = bacc.Bacc("TRN2", target_bir_lowering=False, debug=False,
                   num_devices=N_CORES)

    xb = nc.dram_tensor("xb", [128, NDC, 2, S], F8, kind="ExternalInput")
    wqb = nc.dram_tensor("wqb", [128, NDC, 2, QW], F8, kind="ExternalInput")
    wkb = nc.dram_tensor("wkb", [128, NDC, 2, HD], F8, kind="ExternalInput")
    wvb = nc.dram_tensor("wvb", [128, NDC, 2, HD], F8, kind="ExternalInput")
    wob = nc.dram_tensor("wob", [128, 2, HPC, D], F8, kind="ExternalInput")
    cosT = nc.dram_tensor("cosT", [HD, S], BF, kind="ExternalInput")
    sinT = nc.dram_tensor("sinT", [HD, S], BF, kind="ExternalInput")
    rmat = nc.dram_tensor("rmat", [HD, HD], BF, kind="ExternalInput")
    masks = nc.dram_tensor("masks", [128, 512], BF, kind="ExternalInput")
    out = nc.dram_tensor("out", [S, D], BF, kind="ExternalOutput")

    with tile.TileContext(nc) as tc:
        _emit(tc, nc, xb, wqb, wkb, wvb, wob,
              cosT, sinT, rmat, masks, out)
    nc.compile()
    return nc


def _emit(tc, nc, xb, wqb, wkb, wvb, wob,
          cosT, sinT, rmat, masks, out):
    import contextlib
    ctx = contextlib.ExitStack()
    with ctx:
        const = ctx.enter_context(tc.tile_pool(name="const", bufs=1))
        work = ctx.enter_context(tc.tile_pool(name="work", bufs=1))
        tmp = ctx.enter_context(tc.tile_pool(name="tmp", bufs=4))
        pt_pool = ctx.enter_context(tc.tile_pool(name="pt", bufs=8))
        outp = ctx.enter_context(tc.tile_pool(name="outp", bufs=3))
        ps = ctx.enter_context(tc.tile_pool(name="ps", bufs=8, space="PSUM"))

        # ---- DMA emission, ordered to pace the chunk-major PE stream ----
        rmat_sb = const.tile([HD, HD], BF, tag="rmat")
        ones_sb = const.tile([128, 1], BF, tag="ones")
        nc.vector.memset(ones_sb[:], 1.0)
        ebias = const.tile([128, 1], F32, tag="ebias")
        nc.vector.memset(ebias[:], EXP_SHIFT)

        wk_sb = const.tile([128, NDC, 2, HD], F8, tag="wk")   # dim2: hi/lo
        nc.sync.dma_start(out=rmat_sb[:], in_=rmat.ap())

        gx4, gq4 = {}, {}
        for c in range(0, NDC, 4):
            gx4[c] = const.tile([128, 4, 2, S], F8, tag=f"x{c//4}", name=f"x{c//4}")
        gxh = {c: gx4[c - c % 4][:, c % 4:c % 4 + 2, 0, :] for c in range(0, NDC, 2)}
        gxl = {c: gx4[c - c % 4][:, c % 4:c % 4 + 2, 1, :] for c in range(0, NDC, 2)}
        gqh, gql = {}, {}
        # half-0 of x, wq, and wk interleaved in consumption order
        for c in range(0, NDC, 4):
            nc.sync.dma_start(out=wk_sb[:, c:c + 4, :, :],
                              in_=wkb.ap()[:, c:c + 4, :, :])
            g = const.tile([128, 4, 2, QW], F8, tag=f"q{c//4}", name=f"q{c//4}")
            nc.sync.dma_start(out=g[:], in_=wqb.ap()[:, c:c + 4, :, :])
            gqh[c], gqh[c + 2] = g[:, 0:2, 0, :], g[:, 2:4, 0, :]
            gql[c], gql[c + 2] = g[:, 0:2, 1, :], g[:, 2:4, 1, :]
            nc.sync.dma_start(out=gx4[c][:, :, :, 0:512],
                              in_=xb.ap()[:, c:c + 4, :, 0:512])
        cos_sb = const.tile([HD, S], BF, tag="cos")
        nc.sync.dma_start(out=cos_sb[:], in_=cosT.ap())
        sin_sb = const.tile([HD, S], BF, tag="sin")
        nc.sync.dma_start(out=sin_sb[:], in_=sinT.ap())
        # half-1 of x
        for c in range(0, NDC, 4):
            nc.sync.dma_start(out=gx4[c][:, :, :, 512:S],
                              in_=xb.ap()[:, c:c + 4, :, 512:S])
        wv_sb = const.tile([128, NDC, 2, HD], F8, tag="wv")
        nc.sync.dma_start(out=wv_sb[:], in_=wvb.ap())
        mask_sb = const.tile([128, 512], BF, tag="mask")
        nc.sync.dma_start(out=mask_sb[:], in_=masks.ap())
        wo_sb = const.tile([128, 2, HPC, D], F8, tag="wo")    # dim1: hi/lo
        for n in range(2):
            sl = slice(n * 2048, (n + 1) * 2048)
            nc.sync.dma_start(out=wo_sb[:, :, :, sl], in_=wob.ap()[:, :, :, sl])

        # persistent activations
        khat = work.tile([HD, S], BF, tag="khat")
        qhat = [work.tile([HD, S], BF, tag=f"qhat{h}", name=f"qhat{h}")
                for h in range(HPC)]
        v_sb = [work.tile([128, HD], BF, tag=f"v{i}", name=f"v{i}")
                for i in range(8)]
        ctx_hi = [work.tile([128, 2, S], F8, tag=f"cth{u}", name=f"cth{u}")
                  for u in range(2)]
        ctx_lo = [work.tile([128, 2, S], F8, tag=f"ctl{u}", name=f"ctl{u}")
                  for u in range(2)]

        # ---- K+Q projections: chunk-major across 5 chains per s-quarter ----
        # Per chunk pair, all five tensors advance their 3-term DoubleRow
        # chains, so the PE stream follows the x/wq DMA arrival order.
        # RoPE for each finished s-half is queued and its PE/Act/DVE work is
        # injected into later quarters' streams (and the v-projection).
        TENS = [("k", khat, lambda c: wk_sb[:, c:c + 2, 0, :],
                 lambda c: wk_sb[:, c:c + 2, 1, :], 1.0 / (XS * WKS))]
        for h in range(HPC):
            hsl = slice(h * HD, (h + 1) * HD)
            TENS.append((f"q{h}", qhat[h],
                         lambda c, s=hsl: gqh[c][:, :, s],
                         lambda c, s=hsl: gql[c][:, :, s], 1.0 / (XS * WQS)))
        raws = {ti: work.tile([HD, S], BF, tag=f"raw{ti}", name=f"raw{ti}")
                for ti in range(5)}

        pend = []   # queued rope-finish closures (one per (tensor, half))

        def inject_rope():
            if pend:
                pend.pop(0)()

        def rope_half(ti, half):
            name, dst, _, _, _ = TENS[ti]
            sl = slice(half * 512, (half + 1) * 512)
            t1 = tmp.tile([HD, 512], BF, tag="rope_t1", name="rope_t1", bufs=2)
            nc.vector.tensor_mul(t1[:], raws[ti][:, sl], cos_sb[:, sl])
            rq = ps.tile([HD, 512], F32, tag="ps", name="rq")
            nc.tensor.matmul(rq[:], rmat_sb[:], raws[ti][:, sl],
                             start=True, stop=True)
            rqs = tmp.tile([HD, 512], BF, tag="rope_rqs", name="rope_rqs", bufs=2)
            nc.scalar.activation(rqs[:], rq[:],
                                 mybir.ActivationFunctionType.Copy)
            t2 = tmp.tile([HD, 512], BF, tag="rope_t2", name="rope_t2", bufs=2)
            nc.vector.tensor_mul(t2[:], rqs[:], sin_sb[:, sl])
            nc.vector.tensor_add(dst[:, sl], t1[:], t2[:])

        def v_terms(vpsum, i, p):
            c = 2 * p
            tsl = slice(i * 128, (i + 1) * 128)
            nc.tensor.matmul(vpsum[:], gxh[c][:, :, tsl], wv_sb[:, c:c + 2, 0, :],
                             start=(p == 0), stop=False,
                             perf_mode=DR, skip_group_check=True)
            nc.tensor.matmul(vpsum[:], gxh[c][:, :, tsl], wv_sb[:, c:c + 2, 1, :],
                             start=False, stop=False,
                             perf_mode=DR, skip_group_check=True)
            nc.tensor.matmul(vpsum[:], gxl[c][:, :, tsl], wv_sb[:, c:c + 2, 0, :],
                             start=False, stop=(p == NP - 1),
                             perf_mode=DR, skip_group_check=True)

        for half in (0, 1):
            hoff = half * 512
            chains = [ps.tile([128, 512], F32, tag="ps", name=f"ch{ti}")
                      for ti in range(5)]
            for p in range(NP):
                c = 2 * p
                for term in range(3):
                    for ti, (_, _, whi, wlo, _) in enumerate(TENS):
                        pp = chains[ti]
                        w = whi(c) if term != 1 else wlo(c)
                        for q in range(2):
                            ssl = slice(hoff + q * 256, hoff + (q + 1) * 256)
                            osl = slice(q * 256, (q + 1) * 256)
                            xop = gxl[c] if term == 2 else gxh[c]
                            nc.tensor.matmul(
                                pp[:, osl], w, xop[:, :, ssl],
                                start=(p == 0 and term == 0 and q == 0),
                                stop=(p == NP - 1 and term == 2 and q == 1),
                                perf_mode=DR, skip_group_check=True)
                if p in (4, 9, 14):
                    inject_rope()
            for ti, (_, _, _, _, descale) in enumerate(TENS):
                nc.scalar.activation(raws[ti][:, hoff:hoff + 512], chains[ti][:],
                                     mybir.ActivationFunctionType.Copy,
                                     scale=descale)
            for ti in range(5):
                pend.append(lambda t=ti, hf=half: rope_half(t, hf))

        # ---- V projection: emitted as PE filler inside early attention ----
        def v_chain(i):
            tsl = slice(i * 128, (i + 1) * 128)
            vp = ps.tile([128, HD], F32, tag="ps", name="vp")
            for p in range(NP):
                c = 2 * p
                nc.tensor.matmul(vp[:], gxh[c][:, :, tsl], wv_sb[:, c:c + 2, 0, :],
                                 start=(p == 0), stop=False, perf_mode=DR)
            for p in range(NP):
                c = 2 * p
                nc.tensor.matmul(vp[:], gxh[c][:, :, tsl], wv_sb[:, c:c + 2, 1, :],
                                 start=False, stop=False, perf_mode=DR)
            for p in range(NP):
                c = 2 * p
                nc.tensor.matmul(vp[:], gxl[c][:, :, tsl], wv_sb[:, c:c + 2, 0, :],
                                 start=False, stop=(p == NP - 1), perf_mode=DR)
            nc.scalar.activation(v_sb[i][:], vp[:],
                                 mybir.ActivationFunctionType.Copy,
                                 scale=1.0 / (XS * WVS))
            inject_rope()

        v_chain(0)
        v_chain(1)
        vq = list(range(2, 8))
        while pend and len(vq) > 4:
            v_chain(vq.pop(0))
        while pend:
            inject_rope()

        # ---- attention + out-proj, software-pipelined ----
        def emit_scores(h, b):
            ssl = slice(b * 256, (b + 1) * 256)
            pts = []
            for tp in range(b + 1):
                st = ps.tile([128, 512], F32, tag="ps", name="st")
                for i in range(2):
                    t0 = (2 * tp + i) * 128
                    nc.tensor.matmul(st[:, i * 256:(i + 1) * 256],
                                     khat[:, t0:t0 + 128], qhat[h][:, ssl],
                                     start=True, stop=True)
                pt = pt_pool.tile([128, 512], BF, tag="pt", name="pt")
                nc.scalar.activation(pt[:], st[:],
                                     mybir.ActivationFunctionType.Exp,
                                     bias=ebias[:])
                if tp == b:
                    nc.vector.tensor_mul(pt[:], pt[:], mask_sb[:])
                pts.append(pt)
            return pts

        def emit_denctx(h, b, pts):
            ssl = slice(b * 256, (b + 1) * 256)
            den = ps.tile([1, 256], F32, tag="ps", name="den")
            n_mm = 2 * (b + 1)
            k = 0
            for pt in pts:
                for i in range(2):
                    nc.tensor.matmul(den[:], ones_sb[:],
                                     pt[:, i * 256:(i + 1) * 256],
                                     start=(k == 0), stop=(k == n_mm - 1))
                    k += 1
            cx = ps.tile([HD, 256], F32, tag="ps", name="cx")
            k = 0
            for tp, pt in enumerate(pts):
                for i in range(2):
                    nc.tensor.matmul(cx[:], v_sb[2 * tp + i][:],
                                     pt[:, i * 256:(i + 1) * 256],
                                     start=(k == 0), stop=(k == n_mm - 1))
                    k += 1
            rec = tmp.tile([1, 256], F32, tag="rec", name="rec", bufs=2)
            nc.vector.reciprocal(rec[:], den[:])
            bc = tmp.tile([128, 256], F32, tag="bc", name="bc", bufs=2)
            nc.gpsimd.partition_broadcast(bc[:], rec[:])
            ctxn = tmp.tile([HD, 256], F32, tag="ctxn", name="ctxn", bufs=2)
            nc.vector.scalar_tensor_tensor(
                ctxn[:], cx[:], CTXS, bc[:],
                op0=mybir.AluOpType.mult, op1=mybir.AluOpType.mult)
            u, par = divmod(h, 2)
            nc.scalar.activation(ctx_hi[u][:, par, ssl], ctxn[:],
                                 mybir.ActivationFunctionType.Copy)
            nc.vector.tensor_sub(ctx_lo[u][:, par, ssl], ctxn[:],
                                 ctx_hi[u][:, par, ssl])

        descale = 1.0 / (CTXS * WOS)

        def emit_outproj(t8):
            tsl = slice(t8 * 128, (t8 + 1) * 128)
            for n4 in range(4):
                ot = outp.tile([128, 1024], BF, tag="ot", name="ot")
                for sub in range(4):
                    n = 4 * n4 + sub
                    nsl = slice(n * 256, (n + 1) * 256)
                    op = ps.tile([128, 256], F32, tag="ps", name="op")
                    k = 0
                    for u in range(2):
                        for chi, whi in ((ctx_hi, 0), (ctx_lo, 0), (ctx_hi, 1)):
                            nc.tensor.matmul(
                                op[:], chi[u][:, :, tsl],
                                wo_sb[:, whi, 2 * u:2 * u + 2, nsl],
                                start=(k == 0), stop=(k == 5), perf_mode=DR)
                            k += 1
                    eng = nc.vector if (sub % 2 == 0) else nc.gpsimd
                    eng.tensor_scalar_mul(ot[:, sub * 256:(sub + 1) * 256],
                                          op[:], descale)
                nc.sync.dma_start(
                    out=out.ap()[tsl, n4 * 1024:(n4 + 1) * 1024], in_=ot[:])

        stages = [(h, b) for b in range(4) for h in range(HPC)]
        prev = None
        outq = []
        for hb in stages:
            pts = emit_scores(*hb)
            if vq:
                v_chain(vq.pop(0))
            if prev is not None:
                (ph, pb), ppts = prev
                emit_denctx(ph, pb, ppts)
                if ph == HPC - 1:
                    outq.extend([2 * pb, 2 * pb + 1])
            if outq:
                emit_outproj(outq.pop(0))
            prev = (hb, pts)
        (ph, pb), ppts = prev
        emit_denctx(ph, pb, ppts)
        outq.extend([2 * pb, 2 * pb + 1])
        for t8 in outq:
            emit_outproj(t8)


def _prep_inputs(x, cos, sin, Wq, Wk, Wv, Wo):
    """Host-side shard + hi/lo fp8 quantization. Returns per-core inputs."""
    bf = ml_dtypes.bfloat16
    f8 = ml_dtypes.float8_e4m3

    def hilo(a, s):
        hi = np.asarray(a * s, np.float32).astype(f8)
        lo = (np.asarray(a * s, np.float32) - hi.astype(np.float32)).astype(f8)
        return hi, lo

    x2 = np.asarray(x, np.float32).reshape(S, D)
    xTh = np.ascontiguousarray(x2.T).reshape(NDC, 128, S).transpose(1, 0, 2)
    xh_, xl_ = hilo(np.ascontiguousarray(xTh), XS)
    xb_ = np.ascontiguousarray(np.stack([xh_, xl_], axis=2))  # [128,NDC,2,S]

    cosT = np.ascontiguousarray(np.asarray(cos, np.float32).T).astype(bf)
    sinT = np.ascontiguousarray(np.asarray(sin, np.float32).T).astype(bf)

    rmat = np.zeros((HD, HD), np.float32)
    half = HD // 2
    rmat[np.arange(half), np.arange(half) + half] = 1.0
    rmat[np.arange(half) + half, np.arange(half)] = -1.0
    rmat = rmat.astype(bf)

    # diagonal pair mask: keep when t_local (= i*128 + p) <= s_local
    lt = np.arange(128)[:, None]
    ls = np.arange(256)[None, :]
    masks = np.concatenate([(lt + 128 * i <= ls) for i in range(2)], axis=1)
    masks = np.ascontiguousarray(masks).astype(bf)     # [128, 512]

    scale = 1.0 / np.sqrt(np.float32(HD))
    Wq_ = np.asarray(Wq, np.float32) * scale
    Wk_ = np.asarray(Wk, np.float32)
    Wv_ = np.asarray(Wv, np.float32)
    Wo_ = np.asarray(Wo, np.float32)

    def chunked(w):  # [D, m] -> [128, NDC, m]
        m = w.shape[1]
        return np.ascontiguousarray(
            w.reshape(NDC, 128, m).transpose(1, 0, 2))

    in_maps = []
    for r in range(N_CORES):
        wqh_, wql_ = hilo(chunked(Wq_[:, r * QW:(r + 1) * QW]), WQS)
        wqb_ = np.ascontiguousarray(np.stack([wqh_, wql_], axis=2))
        wkh_, wkl_ = hilo(chunked(Wk_[:, r * HD:(r + 1) * HD]), WKS)
        wkb_ = np.ascontiguousarray(np.stack([wkh_, wkl_], axis=2))
        wvh_, wvl_ = hilo(chunked(Wv_[:, r * HD:(r + 1) * HD]), WVS)
        wvb_ = np.ascontiguousarray(np.stack([wvh_, wvl_], axis=2))
        wo_r = np.ascontiguousarray(
            Wo_[r * QW:(r + 1) * QW, :].reshape(HPC, 128, D)
            .transpose(1, 0, 2))
        woh_, wol_ = hilo(wo_r, WOS)
        wob_ = np.ascontiguousarray(np.stack([woh_, wol_], axis=1))
        in_maps.append({
            "xb": xb_, "wqb": wqb_, "wkb": wkb_, "wvb": wvb_, "wob": wob_,
            "cosT": cosT, "sinT": sinT, "rmat": rmat, "masks": masks,
        })
    return in_maps


def get_nc():
    if "nc" not in _CACHE:
        _CACHE["nc"] = _build()
    return _CACHE["nc"]


def kernel(x, mask, cos, sin, Wq, Wk, Wv, Wo):
    nc = get_nc()
    in_maps = _prep_inputs(x, cos, sin, Wq, Wk, Wv, Wo)
    res = run_bass_kernel_spmd(nc, in_maps, core_ids=list(range(N_CORES)))
    acc = np.zeros((S, D), np.float32)
    for r in range(N_CORES):
        acc += res.results[r]["out"].astype(np.float32)
    return acc[None]


if __name__ == "__main__":
    print("built:", get_nc() is not None)


# revision 35
# speedup vs baseline: 1.1600x; 1.0265x over previous
"""Grouped-query attention, tensor-parallel over heads across 8 TRN2 NeuronCores.

Problem (hardcoded): x[1,1024,4096] @ Wq/Wk/Wv -> RoPE -> causal GQA
(32 q heads, 8 kv groups, head_dim 128) -> out proj Wo -> [1,1024,4096].

Sharding: core r owns q heads 4r..4r+3 and kv group r (Wq/Wk/Wv column
shards, Wo row shard). Each core computes a full [1024,4096] partial of
the output projection; the host sums the 8 partials (the "all-reduce").

Device kernel (per core): the big GEMMs (Q/K/V projections, out-proj)
run in fp8e4 DoubleRow mode, which processes two 128-deep contraction
chunks per instruction at half the cycles/row of bf16.  Precision is
recovered with a 3-term hi/lo split quantization (x_hi@W_hi + x_lo@W_hi
+ x_hi@W_lo), where hi and lo shares one power-of-2 scale so all terms
accumulate in a single PSUM chain; measured end-to-end error matches
bf16.  The attention core (scores, exp, denominators, ctx) stays bf16
with 256-wide s-blocks and causal tile skipping.
"""

import numpy as np
import ml_dtypes

import concourse.bass as bass
import concourse.bacc as bacc
import concourse.mybir as mybir
import concourse.tile as tile
import concourse.bass_isa as bass_isa
from concourse.bass_utils import run_bass_kernel_spmd

S = 1024          # sequence length
D = 4096          # model dim
H = 32            # query heads (global)
G = 8             # kv groups (global)
HD = 128          # head dim
N_CORES = 8
HPC = H // N_CORES   # 4 query heads per core
QW = HPC * HD        # 512 q-proj cols per core
NDC = D // 128       # 32 contraction chunks
NP = NDC // 2        # 16 DoubleRow chunk pairs
BF = mybir.dt.bfloat16
F8 = mybir.dt.float8e4
F32 = mybir.dt.float32
DR = mybir.MatmulPerfMode.DoubleRow

# quantization scales (powers of 2; hi and lo share the scale so every
# 3-term matmul accumulates in one PSUM chain)
XS = 16.0
WQS = 8192.0        # applied to Wq/sqrt(HD)
WKS = 1024.0
WVS = 1024.0
WOS = 1024.0
CTXS = 16.0
EXP_SHIFT = -6.0    # exp(s - 6): keeps bf16 P comfortably in range

_CACHE = {}


def _build():
    nc = bacc.Bacc("TRN2", target_bir_lowering=False, debug=False,
                   num_devices=N_CORES)

    xb = nc.dram_tensor("xb", [128, NDC, 2, S], F8, kind="ExternalInput")
    wqb = nc.dram_tensor("wqb", [128, NDC, 2, QW], F8, kind="ExternalInput")
    wkb = nc.dram_tensor("wkb", [128, NDC, 2, HD], F8, kind="ExternalInput")
    wvb = nc.dram_tensor("wvb", [128, NDC, 2, HD], F8, kind="ExternalInput")
    wob = nc.dram_tensor("wob", [128, 2, HPC, D], F8, kind="ExternalInput")
    cosT = nc.dram_tensor("cosT", [HD, S], BF, kind="ExternalInput")
    sinT = nc.dram_tensor("sinT", [HD, S], BF, kind="ExternalInput")
    rmat = nc.dram_tensor("rmat", [HD, HD], BF, kind="ExternalInput")
    masks = nc.dram_tensor("masks", [128, 512], BF, kind="ExternalInput")
    out = nc.dram_tensor("out", [S, D], BF, kind="ExternalOutput")

    with tile.TileContext(nc) as tc:
        _emit(tc, nc, xb, wqb, wkb, wvb, wob,
              cosT, sinT, rmat, masks, out)
    nc.compile()
    return nc


def _emit(tc, nc, xb, wqb, wkb, wvb, wob,
          cosT, sinT, rmat, masks, out):
    import contextlib
    ctx = contextlib.ExitStack()
    with ctx:
        const = ctx.enter_context(tc.tile_pool(name="const", bufs=1))
        work = ctx.enter_context(tc.tile_pool(name="work", bufs=1))
        tmp = ctx.enter_context(tc.tile_pool(name="tmp", bufs=4))
        pt_pool = ctx.enter_context(tc.tile_pool(name="pt", bufs=8))
        outp = ctx.enter_context(tc.tile_pool(name="outp", bufs=2))
        ps = ctx.enter_context(tc.tile_pool(name="ps", bufs=8, space="PSUM"))

        # ---- DMA emission, ordered to pace the chunk-major PE stream ----
        rmat_sb = const.tile([HD, HD], BF, tag="rmat")
        ones_sb = const.tile([128, 1], BF, tag="ones")
        nc.vector.memset(ones_sb[:], 1.0)
        ebias = const.tile([128, 1], F32, tag="ebias")
        nc.vector.memset(ebias[:], EXP_SHIFT)

        wk_sb = const.tile([128, NDC, 2, HD], F8, tag="wk")   # dim2: hi/lo
        nc.sync.dma_start(out=rmat_sb[:], in_=rmat.ap())

        gx4, gq4 = {}, {}
        for c in range(0, NDC, 4):
            gx4[c] = const.tile([128, 4, 2, S], F8, tag=f"x{c//4}", name=f"x{c//4}")
        gxh = {c: gx4[c - c % 4][:, c % 4:c % 4 + 2, 0, :] for c in range(0, NDC, 2)}
        gxl = {c: gx4[c - c % 4][:, c % 4:c % 4 + 2, 1, :] for c in range(0, NDC, 2)}
        gqh, gql = {}, {}
        # half-0 of x, wq, and wk interleaved in consumption order
        for c in range(0, NDC, 4):
            nc.sync.dma_start(out=wk_sb[:, c:c + 4, :, :],
                              in_=wkb.ap()[:, c:c + 4, :, :])
            g = const.tile([128, 4, 2, QW], F8, tag=f"q{c//4}", name=f"q{c//4}")
            gqh[c], gqh[c + 2] = g[:, 0:2, 0, :], g[:, 2:4, 0, :]
            gql[c], gql[c + 2] = g[:, 0:2, 1, :], g[:, 2:4, 1, :]
            if c == 0:
                # fine-grained first group so the PE stream starts early
                for c2 in (0, 2):
                    nc.sync.dma_start(out=g[:, c2:c2 + 2, :, :],
                                      in_=wqb.ap()[:, c2:c2 + 2, :, :])
                    nc.sync.dma_start(out=gx4[0][:, c2:c2 + 2, :, 0:512],
                                      in_=xb.ap()[:, c2:c2 + 2, :, 0:512])
            else:
                nc.sync.dma_start(out=g[:], in_=wqb.ap()[:, c:c + 4, :, :])
                nc.sync.dma_start(out=gx4[c][:, :, :, 0:512],
                                  in_=xb.ap()[:, c:c + 4, :, 0:512])
        cos_sb = const.tile([HD, S], BF, tag="cos")
        nc.sync.dma_start(out=cos_sb[:], in_=cosT.ap())
        sin_sb = const.tile([HD, S], BF, tag="sin")
        nc.sync.dma_start(out=sin_sb[:], in_=sinT.ap())
        # half-1 of x
        for c in range(0, NDC, 4):
            nc.sync.dma_start(out=gx4[c][:, :, :, 512:S],
                              in_=xb.ap()[:, c:c + 4, :, 512:S])
        wv_sb = const.tile([128, NDC, 2, HD], F8, tag="wv")
        nc.sync.dma_start(out=wv_sb[:], in_=wvb.ap())
        mask_sb = const.tile([128, 512], BF, tag="mask")
        nc.sync.dma_start(out=mask_sb[:], in_=masks.ap())
        wo_sb = const.tile([128, 2, HPC, D], F8, tag="wo")    # dim1: hi/lo
        for n in range(2):
            sl = slice(n * 2048, (n + 1) * 2048)
            nc.sync.dma_start(out=wo_sb[:, :, :, sl], in_=wob.ap()[:, :, :, sl])

        # persistent activations
        khat = work.tile([HD, S], BF, tag="khat")
        qhat = [work.tile([HD, S], BF, tag=f"qhat{h}", name=f"qhat{h}")
                for h in range(HPC)]
        v_sb = [work.tile([128, HD], BF, tag=f"v{i}", name=f"v{i}")
                for i in range(8)]
        ctx_hi = [work.tile([128, 2, S], F8, tag=f"cth{u}", name=f"cth{u}")
                  for u in range(2)]
        ctx_lo = [work.tile([128, 2, S], F8, tag=f"ctl{u}", name=f"ctl{u}")
                  for u in range(2)]

        # ---- K+Q projections: chunk-major across 5 chains per s-quarter ----
        # Per chunk pair, all five tensors advance their 3-term DoubleRow
        # chains, so the PE stream follows the x/wq DMA arrival order.
        # RoPE for each finished s-half is queued and its PE/Act/DVE work is
        # injected into later quarters' streams (and the v-projection).
        TENS = [("k", khat, lambda c: wk_sb[:, c:c + 2, 0, :],
                 lambda c: wk_sb[:, c:c + 2, 1, :], 1.0 / (XS * WKS))]
        for h in range(HPC):
            hsl = slice(h * HD, (h + 1) * HD)
            TENS.append((f"q{h}", qhat[h],
                         lambda c, s=hsl: gqh[c][:, :, s],
                         lambda c, s=hsl: gql[c][:, :, s], 1.0 / (XS * WQS)))
        raws = {ti: work.tile([HD, S], BF, tag=f"raw{ti}", name=f"raw{ti}")
                for ti in range(5)}

        pend = []   # queued rope-finish closures (one per (tensor, half))

        def inject_rope():
            if pend:
                pend.pop(0)()

        def rope_half(ti, half):
            name, dst, _, _, _ = TENS[ti]
            sl = slice(half * 512, (half + 1) * 512)
            t1 = tmp.tile([HD, 512], BF, tag="rope_t1", name="rope_t1", bufs=2)
            nc.vector.tensor_mul(t1[:], raws[ti][:, sl], cos_sb[:, sl])
            rq = ps.tile([HD, 512], F32, tag="ps", name="rq")
            nc.tensor.matmul(rq[:], rmat_sb[:], raws[ti][:, sl],
                             start=True, stop=True)
            rqs = tmp.tile([HD, 512], BF, tag="rope_rqs", name="rope_rqs", bufs=2)
            nc.scalar.activation(rqs[:], rq[:],
                                 mybir.ActivationFunctionType.Copy)
            t2 = tmp.tile([HD, 512], BF, tag="rope_t2", name="rope_t2", bufs=2)
            nc.vector.tensor_mul(t2[:], rqs[:], sin_sb[:, sl])
            nc.vector.tensor_add(dst[:, sl], t1[:], t2[:])

        def v_terms(vpsum, i, p):
            c = 2 * p
            tsl = slice(i * 128, (i + 1) * 128)
            nc.tensor.matmul(vpsum[:], gxh[c][:, :, tsl], wv_sb[:, c:c + 2, 0, :],
                             start=(p == 0), stop=False,
                             perf_mode=DR, skip_group_check=True)
            nc.tensor.matmul(vpsum[:], gxh[c][:, :, tsl], wv_sb[:, c:c + 2, 1, :],
                             start=False, stop=False,
                             perf_mode=DR, skip_group_check=True)
            nc.tensor.matmul(vpsum[:], gxl[c][:, :, tsl], wv_sb[:, c:c + 2, 0, :],
                             start=False, stop=(p == NP - 1),
                             perf_mode=DR, skip_group_check=True)

        for half in (0, 1):
            hoff = half * 512
            chains = [ps.tile([128, 512], F32, tag="ps", name=f"ch{ti}")
                      for ti in range(5)]
            for p in range(NP):
                c = 2 * p
                for term in range(3):
                    for ti, (_, _, whi, wlo, _) in enumerate(TENS):
                        pp = chains[ti]
                        w = whi(c) if term != 1 else wlo(c)
                        for q in range(2):
                            ssl = slice(hoff + q * 256, hoff + (q + 1) * 256)
                            osl = slice(q * 256, (q + 1) * 256)
                            xop = gxl[c] if term == 2 else gxh[c]
                            nc.tensor.matmul(
                                pp[:, osl], w, xop[:, :, ssl],
                                start=(p == 0 and term == 0 and q == 0),
                                stop=(p == NP - 1 and term == 2 and q == 1),
                                perf_mode=DR, skip_group_check=True)
                if p in (4, 9, 14):
                    inject_rope()
            for ti, (_, _, _, _, descale) in enumerate(TENS):
                nc.scalar.activation(raws[ti][:, hoff:hoff + 512], chains[ti][:],
                                     mybir.ActivationFunctionType.Copy,
                                     scale=descale)
            for ti in range(5):
                pend.append(lambda t=ti, hf=half: rope_half(t, hf))

        # ---- V projection: emitted as PE filler inside early attention ----
        def v_chain(i):
            tsl = slice(i * 128, (i + 1) * 128)
            vp = ps.tile([128, HD], F32, tag="ps", name="vp")
            for p in range(NP):
                c = 2 * p
                nc.tensor.matmul(vp[:], gxh[c][:, :, tsl], wv_sb[:, c:c + 2, 0, :],
                                 start=(p == 0), stop=False, perf_mode=DR)
            for p in range(NP):
                c = 2 * p
                nc.tensor.matmul(vp[:], gxh[c][:, :, tsl], wv_sb[:, c:c + 2, 1, :],
                                 start=False, stop=False, perf_mode=DR)
            for p in range(NP):
                c = 2 * p
                nc.tensor.matmul(vp[:], gxl[c][:, :, tsl], wv_sb[:, c:c + 2, 0, :],
                                 start=False, stop=(p == NP - 1), perf_mode=DR)
            nc.scalar.activation(v_sb[i][:], vp[:],
                                 mybir.ActivationFunctionType.Copy,
                                 scale=1.0 / (XS * WVS))
            inject_rope()

        v_chain(0)
        v_chain(1)
        vq = list(range(2, 8))
        while pend and len(vq) > 4:
            v_chain(vq.pop(0))
        while pend:
            inject_rope()

        # ---- attention + out-proj, software-pipelined ----
        def emit_scores(h, b):
            ssl = slice(b * 256, (b + 1) * 256)
            pts = []
            for tp in range(b + 1):
                st = ps.tile([128, 512], F32, tag="ps", name="st")
                for i in range(2):
                    t0 = (2 * tp + i) * 128
                    nc.tensor.matmul(st[:, i * 256:(i + 1) * 256],
                                     khat[:, t0:t0 + 128], qhat[h][:, ssl],
                                     start=True, stop=True)
                pt = pt_pool.tile([128, 512], BF, tag="pt", name="pt")
                nc.scalar.activation(pt[:], st[:],
                                     mybir.ActivationFunctionType.Exp,
                                     bias=ebias[:])
                if tp == b:
                    nc.vector.tensor_mul(pt[:], pt[:], mask_sb[:])
                pts.append(pt)
            return pts, None

        def emit_denctx(h, b, pts, dacc):
            ssl = slice(b * 256, (b + 1) * 256)
            den = ps.tile([1, 256], F32, tag="ps", name="den")
            n_mm = 2 * (b + 1)
            k = 0
            for pt in pts:
                for i in range(2):
                    nc.tensor.matmul(den[:], ones_sb[:],
                                     pt[:, i * 256:(i + 1) * 256],
                                     start=(k == 0), stop=(k == n_mm - 1))
                    k += 1
            cx = ps.tile([HD, 256], F32, tag="ps", name="cx")
            k = 0
            for tp, pt in enumerate(pts):
                for i in range(2):
                    nc.tensor.matmul(cx[:], v_sb[2 * tp + i][:],
                                     pt[:, i * 256:(i + 1) * 256],
                                     start=(k == 0), stop=(k == n_mm - 1))
                    k += 1
            rec = tmp.tile([1, 256], F32, tag="rec", name="rec", bufs=2)
            nc.vector.reciprocal(rec[:], den[:])
            bc = tmp.tile([128, 256], F32, tag="bc", name="bc", bufs=2)
            nc.gpsimd.partition_broadcast(bc[:], rec[:])
            ctxn = tmp.tile([HD, 256], F32, tag="ctxn", name="ctxn", bufs=2)
            nc.vector.scalar_tensor_tensor(
                ctxn[:], cx[:], CTXS, bc[:],
                op0=mybir.AluOpType.mult, op1=mybir.AluOpType.mult)
            u, par = divmod(h, 2)
            nc.scalar.activation(ctx_hi[u][:, par, ssl], ctxn[:],
                                 mybir.ActivationFunctionType.Copy)
            nc.vector.tensor_sub(ctx_lo[u][:, par, ssl], ctxn[:],
                                 ctx_hi[u][:, par, ssl])

        descale = 1.0 / (CTXS * WOS)

        def emit_outproj(t8):
            tsl = slice(t8 * 128, (t8 + 1) * 128)
            for n2 in range(2):
                ot = outp.tile([128, 2048], BF, tag="ot", name="ot")
                for half2 in range(4):
                    op = ps.tile([128, 512], F32, tag="ps", name="op")
                    for sub2 in range(2):
                        n = 8 * n2 + 2 * half2 + sub2
                        nsl = slice(n * 256, (n + 1) * 256)
                        osl = slice(sub2 * 256, (sub2 + 1) * 256)
                        k = 0
                        for u in range(2):
                            for chi, whi in ((ctx_hi, 0), (ctx_lo, 0), (ctx_hi, 1)):
                                nc.tensor.matmul(
                                    op[:, osl], chi[u][:, :, tsl],
                                    wo_sb[:, whi, 2 * u:2 * u + 2, nsl],
                                    start=(k == 0 and sub2 == 0),
                                    stop=(k == 5 and sub2 == 1),
                                    perf_mode=DR, skip_group_check=True)
                                k += 1
                    osl2 = slice(half2 * 512, (half2 + 1) * 512)
                    if half2 % 2 == 0:
                        nc.vector.tensor_scalar_mul(ot[:, osl2], op[:], descale)
                    else:
                        nc.scalar.activation(ot[:, osl2], op[:],
                                             mybir.ActivationFunctionType.Copy,
                                             scale=descale)
                nc.sync.dma_start(
                    out=out.ap()[tsl, n2 * 2048:(n2 + 1) * 2048], in_=ot[:])

        stages = [(h, b) for b in range(4) for h in range(HPC)]
        prev = None
        outq = []
        for hb in stages:
            pts, dacc = emit_scores(*hb)
            if vq:
                v_chain(vq.pop(0))
            todo = outq.pop(0) if outq else None
            if prev is not None:
                (ph, pb), ppts, pdacc = prev
                emit_denctx(ph, pb, ppts, pdacc)
                if ph == HPC - 1:
                    outq.extend([2 * pb, 2 * pb + 1])
            if todo is not None:
                emit_outproj(todo)
            prev = (hb, pts, dacc)
        (ph, pb), ppts, pdacc = prev
        emit_denctx(ph, pb, ppts, pdacc)
        outq.extend([2 * pb, 2 * pb + 1])
        for t8 in outq:
            emit_outproj(t8)


def _prep_inputs(x, cos, sin, Wq, Wk, Wv, Wo):
    """Host-side shard + hi/lo fp8 quantization. Returns per-core inputs."""
    bf = ml_dtypes.bfloat16
    f8 = ml_dtypes.float8_e4m3

    def hilo(a, s):
        hi = np.asarray(a * s, np.float32).astype(f8)
        lo = (np.asarray(a * s, np.float32) - hi.astype(np.float32)).astype(f8)
        return hi, lo

    x2 = np.asarray(x, np.float32).reshape(S, D)
    xTh = np.ascontiguousarray(x2.T).reshape(NDC, 128, S).transpose(1, 0, 2)
    xh_, xl_ = hilo(np.ascontiguousarray(xTh), XS)
    xb_ = np.ascontiguousarray(np.stack([xh_, xl_], axis=2))  # [128,NDC,2,S]

    cosT = np.ascontiguousarray(np.asarray(cos, np.float32).T).astype(bf)
    sinT = np.ascontiguousarray(np.asarray(sin, np.float32).T).astype(bf)

    rmat = np.zeros((HD, HD), np.float32)
    half = HD // 2
    rmat[np.arange(half), np.arange(half) + half] = 1.0
    rmat[np.arange(half) + half, np.arange(half)] = -1.0
    rmat = rmat.astype(bf)

    # diagonal pair mask: keep when t_local (= i*128 + p) <= s_local
    lt = np.arange(128)[:, None]
    ls = np.arange(256)[None, :]
    masks = np.concatenate([(lt + 128 * i <= ls) for i in range(2)], axis=1)
    masks = np.ascontiguousarray(masks).astype(bf)     # [128, 512]

    scale = 1.0 / np.sqrt(np.float32(HD))
    Wq_ = np.asarray(Wq, np.float32) * scale
    Wk_ = np.asarray(Wk, np.float32)
    Wv_ = np.asarray(Wv, np.float32)
    Wo_ = np.asarray(Wo, np.float32)

    def chunked(w):  # [D, m] -> [128, NDC, m]
        m = w.shape[1]
        return np.ascontiguousarray(
            w.reshape(NDC, 128, m).transpose(1, 0, 2))

    in_maps = []
    for r in range(N_CORES):
        wqh_, wql_ = hilo(chunked(Wq_[:, r * QW:(r + 1) * QW]), WQS)
        wqb_ = np.ascontiguousarray(np.stack([wqh_, wql_], axis=2))
        wkh_, wkl_ = hilo(chunked(Wk_[:, r * HD:(r + 1) * HD]), WKS)
        wkb_ = np.ascontiguousarray(np.stack([wkh_, wkl_], axis=2))
        wvh_, wvl_ = hilo(chunked(Wv_[:, r * HD:(r + 1) * HD]), WVS)
        wvb_ = np.ascontiguousarray(np.stack([wvh_, wvl_], axis=2))
        wo_r = np.ascontiguousarray(
            Wo_[r * QW:(r + 1) * QW, :].reshape(HPC, 128, D)
            .transpose(1, 0, 2))
        woh_, wol_ = hilo(wo_r, WOS)
        wob_ = np.ascontiguousarray(np.stack([woh_, wol_], axis=1))
        in_maps.append({
            "xb": xb_, "wqb": wqb_, "wkb": wkb_, "wvb": wvb_, "wob": wob_,
            "cosT": cosT, "sinT": sinT, "rmat": rmat, "masks": masks,
        })
    return in_maps


def get_nc():
    if "nc" not in _CACHE:
        _CACHE["nc"] = _build()
    return _CACHE["nc"]


def kernel(x, mask, cos, sin, Wq, Wk, Wv, Wo):
    nc = get_nc()
    in_maps = _prep_inputs(x, cos, sin, Wq, Wk, Wv, Wo)
    res = run_bass_kernel_spmd(nc, in_maps, core_ids=list(range(N_CORES)))
    acc = np.zeros((S, D), np.float32)
    for r in range(N_CORES):
        acc += res.results[r]["out"].astype(np.float32)
    return acc[None]


if __name__ == "__main__":
    print("built:", get_nc() is not None)
